# revision 25
# baseline (speedup 1.0000x reference)
"""Trainium2 Bass kernel for CompressiveMemory (Infini-attention style).

Sharding: 8 cores = 4 batch x 2 head-groups (8 heads each). The reference's
`att.reshape(B, SEG, H*dv)` is a raw view of a (B,H,SEG,dv) tensor, so each
block of 16 output rows depends on exactly one head: head-sharding needs no
cross-core reduction, only row scattering (done on host).

Per-core kernel (all layouts chosen so matmul contractions land on the
partition dim, avoiding transposes except 2 tiny ones per head-segment):
  - host passes x[b] pre-transposed/tiled as xt[t, p, c, s]
  - per segment: project qT,kT (dk-on-partitions) and v (natural), elu+1
  - attention computed transposed: scoresT = kT.T-contracted, exp without
    max-subtraction (scores are O(1): q,k ~ N(0,1), scaled by 1/8), softmax
    denominator via ones-matmul partition reduction
  - memory state [dk, dv+1] per head (fp32 master + compute-dtype shadow)
  - output projection uses the raw-view structure: 16 rank-64 PSUM-accumulated
    matmuls whose lhsT operands are pure AP slices of the transposed att tile
"""

import os
import sys

for _p in ("/opt/trn_rl_repo",):
    if _p not in sys.path and os.path.isdir(_p):
        sys.path.insert(0, _p)

from contextlib import ExitStack

import ml_dtypes
import numpy as np

import concourse.bass as bass
import concourse.tile as tile
from concourse import bacc, mybir
from concourse.bass_utils import run_bass_kernel_spmd

AF = mybir.ActivationFunctionType
OP = mybir.AluOpType
F32 = mybir.dt.float32

B, S, D = 4, 8192, 1024
H, dk, dv, SEG = 16, 64, 64, 256
HL = 8  # heads per core
NCORES = 8

NSEG = int(os.environ.get("BASS_NSEG", S // SEG))
USE_BF16 = os.environ.get("BASS_CDT", "bf16") == "bf16"
CDT = mybir.dt.bfloat16 if USE_BF16 else F32
NPDT = ml_dtypes.bfloat16 if USE_BF16 else np.float32


def _emit(ctx, tc, nseg, xt_d, wq_d, wk_d, wv_d, wout_d, bsig_d, bsig1m_d, id64_d, out_d):
    nc = tc.nc

    consts = ctx.enter_context(tc.tile_pool(name="consts", bufs=1))
    state_p = ctx.enter_context(tc.tile_pool(name="state", bufs=1))
    xt_p = ctx.enter_context(tc.tile_pool(name="xtp", bufs=2))
    qk_p = ctx.enter_context(tc.tile_pool(name="qk", bufs=2))
    pt_p = ctx.enter_context(tc.tile_pool(name="ptp", bufs=3))
    at_p = ctx.enter_context(tc.tile_pool(name="atp", bufs=2))
    sm_p = ctx.enter_context(tc.tile_pool(name="smp", bufs=3))
    ob_p = ctx.enter_context(tc.tile_pool(name="obp", bufs=2))
    ps = ctx.enter_context(tc.tile_pool(name="ps", bufs=8, space="PSUM"))
    pp = pa = po = ps

    wq_sb = consts.tile([128, 8, 512], CDT, tag="wq")
    wk_sb = consts.tile([128, 8, 512], CDT, tag="wk")
    wv_sb = consts.tile([128, 8, 512], CDT, tag="wv")
    wout_sb = consts.tile([64, 16, 1024], CDT, tag="wout")
    nc.sync.dma_start(wq_sb[:], wq_d.ap())
    nc.sync.dma_start(wk_sb[:], wk_d.ap())
    nc.sync.dma_start(wv_sb[:], wv_d.ap())
    nc.sync.dma_start(wout_sb[:], wout_d.ap())
    bsig_sb = consts.tile([64, HL], F32, tag="bsig")
    bsig1m_sb = consts.tile([64, HL], F32, tag="bsig1m")
    nc.sync.dma_start(bsig_sb[:], bsig_d.ap())
    nc.sync.dma_start(bsig1m_sb[:], bsig1m_d.ap())
    ident = consts.tile([128, 64], F32, tag="ident")
    nc.sync.dma_start(ident[:], id64_d.ap())
    onesB = consts.tile([128, 128], CDT, tag="onesB")
    nc.vector.memset(onesB[:], 1.0)
    ones128c = consts.tile([128, 128], CDT, tag="ones128c")
    nc.vector.memset(ones128c[:], 1.0)

    # per-head memory state, 2 heads packed on partitions: [dk*2, pack, dv+1]
    st32 = state_p.tile([128, HL // 2, 65], F32, tag="st32")
    stcd = state_p.tile([128, HL // 2, 65], CDT, tag="stcd")
    nc.vector.memset(st32[:], 0.0)
    nc.vector.memset(st32[:, :, 64:65], 1.0 / dk)
    nc.scalar.copy(stcd[:], st32[:])

    assert nseg % 2 == 0
    for T in range(nseg // 2):
        # superseg of 2 segments: projections at N=512, halving matmul+ldweights count
        xt_sb = xt_p.tile([128, 8, 2, SEG], CDT, tag="xt")
        nc.sync.dma_start(xt_sb[:], xt_d.ap()[2 * T : 2 * T + 2].rearrange("g p c s -> p c g s"))

        # ---- projections: qT,kT in [dk(2 heads), pack, l=512]; v natural ----
        qt = qk_p.tile([128, 4, 512], CDT, tag="qt")
        kt = qk_p.tile([128, 4, 512], CDT, tag="kt")
        vE = qk_p.tile([128, 4, HL, 65], CDT, tag="vE")
        for w_sb, dst in ((wq_sb, qt), (wk_sb, kt)):
            for pkk in range(4):
                prj = pp.tile([128, 512], F32, tag="ps")
                for kc in range(8):
                    nc.tensor.matmul(
                        prj[:],
                        w_sb[:, kc, pkk * 128 : (pkk + 1) * 128],
                        xt_sb[:, kc, :, :],
                        start=(kc == 0),
                        stop=(kc == 7),
                    )
                nc.vector.tensor_copy(dst[:, pkk, :], prj[:])
        for c in range(4):
            prj = pp.tile([128, 512], F32, tag="ps")
            for kc in range(8):
                nc.tensor.matmul(
                    prj[:],
                    xt_sb[:, kc, c // 2, (c % 2) * 128 : (c % 2) * 128 + 128],
                    wv_sb[:, kc, :],
                    start=(kc == 0),
                    stop=(kc == 7),
                )
            nc.vector.tensor_copy(vE[:, c, :, 0:64], prj[:].rearrange("p (h j) -> p h j", h=HL))
        nc.vector.memset(vE[:, :, :, 64:65], 1.0)

        # ---- elu(x)+1 = exp(min(x,0)) + max(x,0) ----
        sq = qk_p.tile([128, 4, 512], CDT, tag="sq")
        sk = qk_p.tile([128, 4, 512], F32, tag="sk")
        for src, dst in ((qt, sq), (kt, sk)):
            m0 = qk_p.tile([128, 4, 512], CDT, tag="m0")
            ex = qk_p.tile([128, 4, 512], CDT, tag="ex")
            nc.gpsimd.tensor_scalar_min(m0[:], src[:], 0.0)
            nc.scalar.activation(ex[:], m0[:], AF.Exp)
            nc.vector.scalar_tensor_tensor(dst[:], src[:], 0.0, ex[:], op0=OP.max, op1=OP.add)

        for s in range(2):
            t = 2 * T + s
            so = s * SEG
            attT = at_p.tile([64, HL, SEG], CDT, tag="attT")

            for h in range(HL):
                hp, pk = h % 2, h // 2
                hs = hp * 64
                qTh = qt[hs : hs + 64, pk, so : so + SEG]
                sqh = sq[hs : hs + 64, pk, so : so + SEG]

                # scoresT[m, l] (2 m-chunks in one bank)
                scT = pa.tile([128, 2, SEG], F32, tag="ps")
                for mc in range(2):
                    nc.tensor.matmul(
                        scT[:, mc, :],
                        kt[hs : hs + 64, pk, so + mc * 128 : so + (mc + 1) * 128],
                        qTh,
                        start=True,
                        stop=True,
                    )
                # P^T = exp(scores/8) (no max subtraction; scores are O(1))
                PT = pt_p.tile([128, 2, SEG], CDT, tag="PT")
                nc.scalar.activation(PT[:], scT[:], AF.Exp, scale=0.125)

                # zBt: z broadcast along free, at this head's partitions
                zBt = sm_p.tile([128, 128], CDT, tag="zBt")
                nc.gpsimd.tensor_scalar_mul(
                    zBt[hs : hs + 64, :], ones128c[hs : hs + 64, :], st32[hs : hs + 64, pk, 64:65]
                )

                # U: dpaT_raw in [0:64, 0:256]; att_memT_raw in [0:64, 256:512]
                UU = pa.tile([128, 512], F32, tag="ps")
                for mc in range(2):
                    nc.tensor.matmul(
                        UU[0:64, 0:256], vE[:, 2 * s + mc, h, 0:64], PT[:, mc, :], start=(mc == 0), stop=(mc == 1)
                    )
                nc.tensor.matmul(UU[0:64, 256:512], stcd[hs : hs + 64, pk, 0:64], sqh, start=True, stop=True)

                # BB: broadcast denominators: sum_m P in [0:64, 0:256],
                # sigma_q @ z in [0:64, 256:512]
                BB = pa.tile([128, 512], F32, tag="ps")
                for mc in range(2):
                    nc.tensor.matmul(BB[0:64, 0:256], onesB[:, 0:64], PT[:, mc, :], start=(mc == 0), stop=(mc == 1))
                nc.tensor.matmul(BB[0:64, 256:512], zBt[hs : hs + 64, 0:64], sqh, start=True, stop=True)

                # reciprocal of both denominators: one fast custom-DVE op
                rb2 = sm_p.tile([64, 512], F32, tag="rb2")
                nc.vector.reciprocal_approx_fast(rb2[:], BB[0:64, 0:512])

                # combine: att = bsig * att_mem / zden + (1-bsig) * dpa / sden
                bn = sm_p.tile([64, SEG], F32, tag="bn")
                t2 = sm_p.tile([64, SEG], F32, tag="t2")
                nc.vector.scalar_tensor_tensor(
                    bn[:], UU[0:64, 256:512], bsig_sb[:, h : h + 1], rb2[:, 256:512], op0=OP.mult, op1=OP.mult
                )
                nc.vector.scalar_tensor_tensor(
                    t2[:], UU[0:64, 0:256], bsig1m_sb[:, h : h + 1], rb2[:, 0:256], op0=OP.mult, op1=OP.mult
                )
                nc.gpsimd.tensor_add(attT[:, h, :], bn[:], t2[:])

                # state update: mem += sigma_k^T v ; z += sum_l sigma_k
                trd = pa.tile([128, 256], F32, tag="ps")
                for mc in range(2):
                    nc.tensor.transpose(
                        trd[:, mc * 64 : (mc + 1) * 64],
                        sk[hs : hs + 64, pk, so + mc * 128 : so + (mc + 1) * 128],
                        ident[hs : hs + 64, 0:64],
                    )
                skn = sm_p.tile([128, 2, 64], CDT, tag="skn")
                for mc in range(2):
                    nc.scalar.copy(skn[:, mc, :], trd[:, mc * 64 : (mc + 1) * 64])
                for mc in range(2):
                    nc.tensor.matmul(
                        trd[hs : hs + 64, 128:193], skn[:, mc, :], vE[:, 2 * s + mc, h, :], start=(mc == 0), stop=(mc == 1)
                    )
                nc.vector.tensor_add(st32[hs : hs + 64, pk, :], st32[hs : hs + 64, pk, :], trd[hs : hs + 64, 128:193])
                nc.scalar.copy(stcd[hs : hs + 64, pk, :], st32[hs : hs + 64, pk, :])

            # ---- output projection: out_rows = att_view @ Wout ----
            o_sb = ob_p.tile([128, 1024], F32, tag="o")
            aT = attT[:].rearrange("p h (lh lm) -> p h lh lm", lm=16)
            for nh in range(2):
                pot = po.tile([128, 512], F32, tag="ps")
                for lm in range(16):
                    rhs = wout_sb[:, lm, nh * 512 : (nh + 1) * 512]
                    nc.tensor.matmul(pot[:], aT[:, :, :, lm], rhs, start=(lm == 0), stop=(lm == 15))
                nc.scalar.copy(o_sb[:, nh * 512 : (nh + 1) * 512], pot[:])
            nc.sync.dma_start(out_d.ap()[t * 128 : (t + 1) * 128, :], o_sb[:])


def build_program(nseg=NSEG):
    nc = bacc.Bacc("TRN2", target_bir_lowering=False, debug=False, num_devices=NCORES)
    xt_d = nc.dram_tensor("xt", [nseg, 128, 8, SEG], CDT, kind="ExternalInput")
    wq_d = nc.dram_tensor("wq", [128, 8, 512], CDT, kind="ExternalInput")
    wk_d = nc.dram_tensor("wk", [128, 8, 512], CDT, kind="ExternalInput")
    wv_d = nc.dram_tensor("wv", [128, 8, 512], CDT, kind="ExternalInput")
    wout_d = nc.dram_tensor("wout", [64, 16, 1024], CDT, kind="ExternalInput")
    bsig_d = nc.dram_tensor("bsig", [64, HL], F32, kind="ExternalInput")
    bsig1m_d = nc.dram_tensor("bsig1m", [64, HL], F32, kind="ExternalInput")
    id64_d = nc.dram_tensor("id64", [128, 64], F32, kind="ExternalInput")
    out_d = nc.dram_tensor("out", [nseg * 128, 1024], F32, kind="ExternalOutput")
    with tile.TileContext(nc) as tc:
        with ExitStack() as ctx:
            _emit(ctx, tc, nseg, xt_d, wq_d, wk_d, wv_d, wout_d, bsig_d, bsig1m_d, id64_d, out_d)
    nc.compile()
    return nc


def shard_inputs(x, Wq, Wk, Wv, Wout, betas, nseg=NSEG):
    x = np.asarray(x, np.float32)
    Wq = np.asarray(Wq, np.float32)
    Wk = np.asarray(Wk, np.float32)
    Wv = np.asarray(Wv, np.float32)
    Wout = np.asarray(Wout, np.float32)
    betas = np.asarray(betas, np.float32)
    sig = 1.0 / (1.0 + np.exp(-betas[0, :, 0, :]))  # [H, dv]

    wout_t = np.ascontiguousarray(Wout.reshape(16, 64, 1024).transpose(1, 0, 2)).astype(NPDT)
    id64 = np.tile(np.eye(64, dtype=np.float32), (2, 1))
    in_maps = []
    for c in range(NCORES):
        b, hg = c // 2, c % 2
        hb = hg * HL
        xt = x[b].T.reshape(8, 128, S // SEG, SEG).transpose(2, 1, 0, 3)[:nseg]
        m = {
            "xt": np.ascontiguousarray(xt).astype(NPDT),
            "wq": np.ascontiguousarray(Wq[:, hb * 64 : (hb + HL) * 64].reshape(8, 128, 512).transpose(1, 0, 2)).astype(NPDT),
            "wk": np.ascontiguousarray(Wk[:, hb * 64 : (hb + HL) * 64].reshape(8, 128, 512).transpose(1, 0, 2)).astype(NPDT),
            "wv": np.ascontiguousarray(Wv[:, hb * 64 : (hb + HL) * 64].reshape(8, 128, 512).transpose(1, 0, 2)).astype(NPDT),
            "wout": wout_t,
            "bsig": np.ascontiguousarray(sig[hb : hb + HL].T),
            "bsig1m": np.ascontiguousarray((1.0 - sig)[hb : hb + HL].T),
            "id64": id64,
        }
        in_maps.append(m)
    return in_maps


def assemble_output(results, nseg=NSEG):
    out = np.empty((B, nseg * SEG, D), np.float32)
    o5 = out.reshape(B, nseg, 2, 128, D)
    for c in range(NCORES):
        b, hg = c // 2, c % 2
        o5[b, :, hg] = results[c]["out"].reshape(nseg, 128, D)
    return out


_COMPILED = {}


def _get_program(nseg=NSEG):
    if nseg not in _COMPILED:
        _COMPILED[nseg] = build_program(nseg)
    return _COMPILED[nseg]


def run(x, Wq, Wk, Wv, Wout, betas, nseg=NSEG, trace=False):
    nc = _get_program(nseg)
    in_maps = shard_inputs(x, Wq, Wk, Wv, Wout, betas, nseg)
    res = run_bass_kernel_spmd(nc, in_maps, list(range(NCORES)), trace=trace)
    return assemble_output(res.results, nseg), res.exec_time_ns


def kernel(x, Wq, Wk, Wv, Wout, betas):
    out, _ = run(x, Wq, Wk, Wv, Wout, betas, nseg=NSEG, trace=False)
    return out


# revision 27
# speedup vs baseline: 1.4962x; 1.4962x over previous
"""Trainium2 Bass kernel for CompressiveMemory (Infini-attention style).

Sharding: 8 cores = 4 batch x 2 head-groups (8 heads each). The reference's
`att.reshape(B, SEG, H*dv)` is a raw view of a (B,H,SEG,dv) tensor, so each
block of 16 output rows depends on exactly one head: head-sharding needs no
cross-core reduction, only row scattering (done on host).

Per-core kernel (all layouts chosen so matmul contractions land on the
partition dim, avoiding transposes except 2 tiny ones per head-segment):
  - host passes x[b] pre-transposed/tiled as xt[t, p, c, s]
  - per segment: project qT,kT (dk-on-partitions) and v (natural), elu+1
  - attention computed transposed: scoresT = kT.T-contracted, exp without
    max-subtraction (scores are O(1): q,k ~ N(0,1), scaled by 1/8), softmax
    denominator via ones-matmul partition reduction
  - memory state [dk, dv+1] per head (fp32 master + compute-dtype shadow)
  - output projection uses the raw-view structure: 16 rank-64 PSUM-accumulated
    matmuls whose lhsT operands are pure AP slices of the transposed att tile
"""

import os
import sys

for _p in ("/opt/trn_rl_repo",):
    if _p not in sys.path and os.path.isdir(_p):
        sys.path.insert(0, _p)

from contextlib import ExitStack

import ml_dtypes
import numpy as np

import concourse.bass as bass
import concourse.tile as tile
from concourse import bacc, mybir
from concourse.bass_utils import run_bass_kernel_spmd

AF = mybir.ActivationFunctionType
OP = mybir.AluOpType
F32 = mybir.dt.float32

B, S, D = 4, 8192, 1024
H, dk, dv, SEG = 16, 64, 64, 256
HL = 8  # heads per core
NCORES = 8

NSEG = int(os.environ.get("BASS_NSEG", S // SEG))
USE_BF16 = os.environ.get("BASS_CDT", "bf16") == "bf16"
CDT = mybir.dt.bfloat16 if USE_BF16 else F32
NPDT = ml_dtypes.bfloat16 if USE_BF16 else np.float32


def _emit(ctx, tc, nseg, xt_d, wq_d, wk_d, wv_d, wout_d, bsig_d, bsig1m_d, id64_d, out_d):
    nc = tc.nc

    consts = ctx.enter_context(tc.tile_pool(name="consts", bufs=1))
    state_p = ctx.enter_context(tc.tile_pool(name="state", bufs=1))
    xt_p = ctx.enter_context(tc.tile_pool(name="xtp", bufs=2))
    qk_p = ctx.enter_context(tc.tile_pool(name="qk", bufs=2))
    pt_p = ctx.enter_context(tc.tile_pool(name="ptp", bufs=3))
    at_p = ctx.enter_context(tc.tile_pool(name="atp", bufs=2))
    sm_p = ctx.enter_context(tc.tile_pool(name="smp", bufs=3))
    ob_p = ctx.enter_context(tc.tile_pool(name="obp", bufs=2))
    ps = ctx.enter_context(tc.tile_pool(name="ps", bufs=8, space="PSUM"))
    pp = pa = po = ps

    wq_sb = consts.tile([128, 8, 512], CDT, tag="wq")
    wk_sb = consts.tile([128, 8, 512], CDT, tag="wk")
    wv_sb = consts.tile([128, 8, 512], CDT, tag="wv")
    wout_sb = consts.tile([64, 16, 1024], CDT, tag="wout")
    nc.sync.dma_start(wq_sb[:], wq_d.ap())
    nc.sync.dma_start(wk_sb[:], wk_d.ap())
    nc.sync.dma_start(wv_sb[:], wv_d.ap())
    nc.sync.dma_start(wout_sb[:], wout_d.ap())
    bsig_sb = consts.tile([64, HL], F32, tag="bsig")
    bsig1m_sb = consts.tile([64, HL], F32, tag="bsig1m")
    nc.sync.dma_start(bsig_sb[:], bsig_d.ap())
    nc.sync.dma_start(bsig1m_sb[:], bsig1m_d.ap())
    ident = consts.tile([128, 64], F32, tag="ident")
    nc.sync.dma_start(ident[:], id64_d.ap())
    onesB = consts.tile([128, 128], CDT, tag="onesB")
    nc.vector.memset(onesB[:], 1.0)
    ones128c = consts.tile([128, 128], CDT, tag="ones128c")
    nc.vector.memset(ones128c[:], 1.0)

    # per-head memory state, 2 heads packed on partitions: [dk*2, pack, dv+1]
    st32 = state_p.tile([128, HL // 2, 65], F32, tag="st32")
    stcd = state_p.tile([128, HL // 2, 65], CDT, tag="stcd")
    nc.vector.memset(st32[:], 0.0)
    nc.vector.memset(st32[:, :, 64:65], 1.0 / dk)
    nc.scalar.copy(stcd[:], st32[:])

    assert nseg % 2 == 0
    for T in range(nseg // 2):
        # superseg of 2 segments: projections at N=512, halving matmul+ldweights count
        xt_sb = xt_p.tile([128, 8, 2, SEG], CDT, tag="xt")
        nc.sync.dma_start(xt_sb[:], xt_d.ap()[2 * T : 2 * T + 2].rearrange("g p c s -> p c g s"))

        # ---- projections: qT,kT in [dk(2 heads), pack, l=512]; v natural ----
        qt = qk_p.tile([128, 4, 512], CDT, tag="qt")
        kt = qk_p.tile([128, 4, 512], CDT, tag="kt")
        vE = qk_p.tile([128, 4, HL, 65], CDT, tag="vE")
        for w_sb, dst in ((wq_sb, qt), (wk_sb, kt)):
            for pkk in range(4):
                prj = pp.tile([128, 512], F32, tag="ps")
                for kc in range(8):
                    nc.tensor.matmul(
                        prj[:],
                        w_sb[:, kc, pkk * 128 : (pkk + 1) * 128],
                        xt_sb[:, kc, :, :],
                        start=(kc == 0),
                        stop=(kc == 7),
                    )
                nc.vector.tensor_copy(dst[:, pkk, :], prj[:])
        for c in range(4):
            prj = pp.tile([128, 512], F32, tag="ps")
            for kc in range(8):
                nc.tensor.matmul(
                    prj[:],
                    xt_sb[:, kc, c // 2, (c % 2) * 128 : (c % 2) * 128 + 128],
                    wv_sb[:, kc, :],
                    start=(kc == 0),
                    stop=(kc == 7),
                )
            nc.vector.tensor_copy(vE[:, c, :, 0:64], prj[:].rearrange("p (h j) -> p h j", h=HL))
        nc.vector.memset(vE[:, :, :, 64:65], 1.0)

        # ---- elu(x)+1 = exp(min(x,0)) + max(x,0) ----
        sq = qk_p.tile([128, 4, 512], CDT, tag="sq")
        sk = qk_p.tile([128, 4, 512], F32, tag="sk")
        for src, dst in ((qt, sq), (kt, sk)):
            m0 = qk_p.tile([128, 4, 512], CDT, tag="m0")
            ex = qk_p.tile([128, 4, 512], CDT, tag="ex")
            nc.vector.tensor_scalar_min(m0[:], src[:], 0.0)
            nc.scalar.activation(ex[:], m0[:], AF.Exp)
            nc.vector.scalar_tensor_tensor(dst[:], src[:], 0.0, ex[:], op0=OP.max, op1=OP.add)

        for s in range(2):
            t = 2 * T + s
            so = s * SEG
            attT = at_p.tile([64, HL, SEG], CDT, tag="attT")

            for h in range(HL):
                hp, pk = h % 2, h // 2
                hs = hp * 64
                qTh = qt[hs : hs + 64, pk, so : so + SEG]
                sqh = sq[hs : hs + 64, pk, so : so + SEG]

                # scoresT[m, l] (2 m-chunks in one bank)
                scT = pa.tile([128, 2, SEG], F32, tag="ps")
                for mc in range(2):
                    nc.tensor.matmul(
                        scT[:, mc, :],
                        kt[hs : hs + 64, pk, so + mc * 128 : so + (mc + 1) * 128],
                        qTh,
                        start=True,
                        stop=True,
                    )
                # P^T = exp(scores/8) (no max subtraction; scores are O(1))
                PT = pt_p.tile([128, 2, SEG], CDT, tag="PT")
                nc.scalar.activation(PT[:], scT[:], AF.Exp, scale=0.125)

                # zBt: z broadcast along free, at this head's partitions
                zBt = sm_p.tile([128, 128], CDT, tag="zBt")
                nc.vector.tensor_scalar_mul(
                    zBt[hs : hs + 64, :], ones128c[hs : hs + 64, :], st32[hs : hs + 64, pk, 64:65]
                )

                # U: dpaT_raw in [0:64, 0:256]; att_memT_raw in [0:64, 256:512]
                UU = pa.tile([128, 512], F32, tag="ps")
                for mc in range(2):
                    nc.tensor.matmul(
                        UU[0:64, 0:256], vE[:, 2 * s + mc, h, 0:64], PT[:, mc, :], start=(mc == 0), stop=(mc == 1)
                    )
                nc.tensor.matmul(UU[0:64, 256:512], stcd[hs : hs + 64, pk, 0:64], sqh, start=True, stop=True)

                # BB: broadcast denominators: sum_m P in [0:64, 0:256],
                # sigma_q @ z in [0:64, 256:512]
                BB = pa.tile([128, 512], F32, tag="ps")
                for mc in range(2):
                    nc.tensor.matmul(BB[0:64, 0:256], onesB[:, 0:64], PT[:, mc, :], start=(mc == 0), stop=(mc == 1))
                nc.tensor.matmul(BB[0:64, 256:512], zBt[hs : hs + 64, 0:64], sqh, start=True, stop=True)

                # reciprocal of both denominators: one fast custom-DVE op
                rb2 = sm_p.tile([64, 512], F32, tag="rb2")
                nc.vector.reciprocal_approx_fast(rb2[:], BB[0:64, 0:512])

                # combine: att = bsig * att_mem / zden + (1-bsig) * dpa / sden
                bn = sm_p.tile([64, SEG], F32, tag="bn")
                t2 = sm_p.tile([64, SEG], F32, tag="t2")
                nc.vector.scalar_tensor_tensor(
                    bn[:], UU[0:64, 256:512], bsig_sb[:, h : h + 1], rb2[:, 256:512], op0=OP.mult, op1=OP.mult
                )
                nc.vector.scalar_tensor_tensor(
                    t2[:], UU[0:64, 0:256], bsig1m_sb[:, h : h + 1], rb2[:, 0:256], op0=OP.mult, op1=OP.mult
                )
                nc.gpsimd.tensor_add(attT[:, h, :], bn[:], t2[:])

                # state update: mem += sigma_k^T v ; z += sum_l sigma_k
                trd = pa.tile([128, 256], F32, tag="ps")
                for mc in range(2):
                    nc.tensor.transpose(
                        trd[:, mc * 64 : (mc + 1) * 64],
                        sk[hs : hs + 64, pk, so + mc * 128 : so + (mc + 1) * 128],
                        ident[hs : hs + 64, 0:64],
                    )
                skn = sm_p.tile([128, 2, 64], CDT, tag="skn")
                for mc in range(2):
                    nc.scalar.copy(skn[:, mc, :], trd[:, mc * 64 : (mc + 1) * 64])
                for mc in range(2):
                    nc.tensor.matmul(
                        trd[hs : hs + 64, 128:193], skn[:, mc, :], vE[:, 2 * s + mc, h, :], start=(mc == 0), stop=(mc == 1)
                    )
                nc.vector.tensor_add(st32[hs : hs + 64, pk, :], st32[hs : hs + 64, pk, :], trd[hs : hs + 64, 128:193])
                nc.scalar.copy(stcd[hs : hs + 64, pk, :], st32[hs : hs + 64, pk, :])

            # ---- output projection: out_rows = att_view @ Wout ----
            o_sb = ob_p.tile([128, 1024], F32, tag="o")
            aT = attT[:].rearrange("p h (lh lm) -> p h lh lm", lm=16)
            for nh in range(2):
                pot = po.tile([128, 512], F32, tag="ps")
                for lm in range(16):
                    rhs = wout_sb[:, lm, nh * 512 : (nh + 1) * 512]
                    nc.tensor.matmul(pot[:], aT[:, :, :, lm], rhs, start=(lm == 0), stop=(lm == 15))
                nc.scalar.copy(o_sb[:, nh * 512 : (nh + 1) * 512], pot[:])
            nc.sync.dma_start(out_d.ap()[t * 128 : (t + 1) * 128, :], o_sb[:])


def build_program(nseg=NSEG):
    nc = bacc.Bacc("TRN2", target_bir_lowering=False, debug=False, num_devices=NCORES)
    xt_d = nc.dram_tensor("xt", [nseg, 128, 8, SEG], CDT, kind="ExternalInput")
    wq_d = nc.dram_tensor("wq", [128, 8, 512], CDT, kind="ExternalInput")
    wk_d = nc.dram_tensor("wk", [128, 8, 512], CDT, kind="ExternalInput")
    wv_d = nc.dram_tensor("wv", [128, 8, 512], CDT, kind="ExternalInput")
    wout_d = nc.dram_tensor("wout", [64, 16, 1024], CDT, kind="ExternalInput")
    bsig_d = nc.dram_tensor("bsig", [64, HL], F32, kind="ExternalInput")
    bsig1m_d = nc.dram_tensor("bsig1m", [64, HL], F32, kind="ExternalInput")
    id64_d = nc.dram_tensor("id64", [128, 64], F32, kind="ExternalInput")
    out_d = nc.dram_tensor("out", [nseg * 128, 1024], F32, kind="ExternalOutput")
    with tile.TileContext(nc) as tc:
        with ExitStack() as ctx:
            _emit(ctx, tc, nseg, xt_d, wq_d, wk_d, wv_d, wout_d, bsig_d, bsig1m_d, id64_d, out_d)
    nc.compile()
    return nc


def shard_inputs(x, Wq, Wk, Wv, Wout, betas, nseg=NSEG):
    x = np.asarray(x, np.float32)
    Wq = np.asarray(Wq, np.float32)
    Wk = np.asarray(Wk, np.float32)
    Wv = np.asarray(Wv, np.float32)
    Wout = np.asarray(Wout, np.float32)
    betas = np.asarray(betas, np.float32)
    sig = 1.0 / (1.0 + np.exp(-betas[0, :, 0, :]))  # [H, dv]

    wout_t = np.ascontiguousarray(Wout.reshape(16, 64, 1024).transpose(1, 0, 2)).astype(NPDT)
    id64 = np.tile(np.eye(64, dtype=np.float32), (2, 1))
    in_maps = []
    for c in range(NCORES):
        b, hg = c // 2, c % 2
        hb = hg * HL
        xt = x[b].T.reshape(8, 128, S // SEG, SEG).transpose(2, 1, 0, 3)[:nseg]
        m = {
            "xt": np.ascontiguousarray(xt).astype(NPDT),
            "wq": np.ascontiguousarray(Wq[:, hb * 64 : (hb + HL) * 64].reshape(8, 128, 512).transpose(1, 0, 2)).astype(NPDT),
            "wk": np.ascontiguousarray(Wk[:, hb * 64 : (hb + HL) * 64].reshape(8, 128, 512).transpose(1, 0, 2)).astype(NPDT),
            "wv": np.ascontiguousarray(Wv[:, hb * 64 : (hb + HL) * 64].reshape(8, 128, 512).transpose(1, 0, 2)).astype(NPDT),
            "wout": wout_t,
            "bsig": np.ascontiguousarray(sig[hb : hb + HL].T),
            "bsig1m": np.ascontiguousarray((1.0 - sig)[hb : hb + HL].T),
            "id64": id64,
        }
        in_maps.append(m)
    return in_maps


def assemble_output(results, nseg=NSEG):
    out = np.empty((B, nseg * SEG, D), np.float32)
    o5 = out.reshape(B, nseg, 2, 128, D)
    for c in range(NCORES):
        b, hg = c // 2, c % 2
        o5[b, :, hg] = results[c]["out"].reshape(nseg, 128, D)
    return out


_COMPILED = {}


def _get_program(nseg=NSEG):
    if nseg not in _COMPILED:
        _COMPILED[nseg] = build_program(nseg)
    return _COMPILED[nseg]


def run(x, Wq, Wk, Wv, Wout, betas, nseg=NSEG, trace=False):
    nc = _get_program(nseg)
    in_maps = shard_inputs(x, Wq, Wk, Wv, Wout, betas, nseg)
    res = run_bass_kernel_spmd(nc, in_maps, list(range(NCORES)), trace=trace)
    return assemble_output(res.results, nseg), res.exec_time_ns


def kernel(x, Wq, Wk, Wv, Wout, betas):
    out, _ = run(x, Wq, Wk, Wv, Wout, betas, nseg=NSEG, trace=False)
    return out


# revision 31
# speedup vs baseline: 1.4980x; 1.0012x over previous
"""Trainium2 Bass kernel for CompressiveMemory (Infini-attention style).

Sharding: 8 cores = 4 batch x 2 head-groups (8 heads each). The reference's
`att.reshape(B, SEG, H*dv)` is a raw view of a (B,H,SEG,dv) tensor, so each
block of 16 output rows depends on exactly one head: head-sharding needs no
cross-core reduction, only row scattering (done on host).

Per-core kernel (all layouts chosen so matmul contractions land on the
partition dim, avoiding transposes except 2 tiny ones per head-segment):
  - host passes x[b] pre-transposed/tiled as xt[t, p, c, s]
  - per segment: project qT,kT (dk-on-partitions) and v (natural), elu+1
  - attention computed transposed: scoresT = kT.T-contracted, exp without
    max-subtraction (scores are O(1): q,k ~ N(0,1), scaled by 1/8), softmax
    denominator via ones-matmul partition reduction
  - memory state [dk, dv+1] per head (fp32 master + compute-dtype shadow)
  - output projection uses the raw-view structure: 16 rank-64 PSUM-accumulated
    matmuls whose lhsT operands are pure AP slices of the transposed att tile
"""

import os
import sys

for _p in ("/opt/trn_rl_repo",):
    if _p not in sys.path and os.path.isdir(_p):
        sys.path.insert(0, _p)

from contextlib import ExitStack

import ml_dtypes
import numpy as np

import concourse.bass as bass
import concourse.tile as tile
from concourse import bacc, mybir
from concourse.bass_utils import run_bass_kernel_spmd

AF = mybir.ActivationFunctionType
OP = mybir.AluOpType
F32 = mybir.dt.float32

B, S, D = 4, 8192, 1024
H, dk, dv, SEG = 16, 64, 64, 256
HL = 8  # heads per core
NCORES = 8

NSEG = int(os.environ.get("BASS_NSEG", S // SEG))
USE_BF16 = os.environ.get("BASS_CDT", "bf16") == "bf16"
CDT = mybir.dt.bfloat16 if USE_BF16 else F32
NPDT = ml_dtypes.bfloat16 if USE_BF16 else np.float32


def _emit(ctx, tc, nseg, xt_d, wq_d, wk_d, wv_d, wout_d, bsig_d, bsig1m_d, id64_d, out_d):
    nc = tc.nc

    consts = ctx.enter_context(tc.tile_pool(name="consts", bufs=1))
    state_p = ctx.enter_context(tc.tile_pool(name="state", bufs=1))
    xt_p = ctx.enter_context(tc.tile_pool(name="xtp", bufs=2))
    qk_p = ctx.enter_context(tc.tile_pool(name="qk", bufs=2))
    pt_p = ctx.enter_context(tc.tile_pool(name="ptp", bufs=4))
    at_p = ctx.enter_context(tc.tile_pool(name="atp", bufs=2))
    sm_p = ctx.enter_context(tc.tile_pool(name="smp", bufs=4))
    ob_p = ctx.enter_context(tc.tile_pool(name="obp", bufs=2))
    ps = ctx.enter_context(tc.tile_pool(name="ps", bufs=8, space="PSUM"))
    pp = pa = po = ps

    wq_sb = consts.tile([128, 8, 512], CDT, tag="wq")
    wk_sb = consts.tile([128, 8, 512], CDT, tag="wk")
    wv_sb = consts.tile([128, 8, 512], CDT, tag="wv")
    wout_sb = consts.tile([64, 16, 1024], CDT, tag="wout")
    nc.sync.dma_start(wq_sb[:], wq_d.ap())
    nc.sync.dma_start(wk_sb[:], wk_d.ap())
    nc.sync.dma_start(wv_sb[:], wv_d.ap())
    nc.sync.dma_start(wout_sb[:], wout_d.ap())
    bsig_sb = consts.tile([64, HL], F32, tag="bsig")
    bsig1m_sb = consts.tile([64, HL], F32, tag="bsig1m")
    nc.sync.dma_start(bsig_sb[:], bsig_d.ap())
    nc.sync.dma_start(bsig1m_sb[:], bsig1m_d.ap())
    ident = consts.tile([128, 64], F32, tag="ident")
    nc.sync.dma_start(ident[:], id64_d.ap())
    onesB = consts.tile([128, 128], CDT, tag="onesB")
    nc.vector.memset(onesB[:], 1.0)
    ones128c = consts.tile([128, 128], CDT, tag="ones128c")
    nc.vector.memset(ones128c[:], 1.0)

    # per-head memory state, 2 heads packed on partitions; one tile per pack
    # (separate tiles so one head-pair's update never serializes another's reads)
    st32s = [state_p.tile([128, 65], F32, tag=f"st32_{i}", name=f"st32_{i}") for i in range(HL // 2)]
    stcds = [state_p.tile([128, 65], CDT, tag=f"stcd_{i}", name=f"stcd_{i}") for i in range(HL // 2)]
    for i in range(HL // 2):
        nc.vector.memset(st32s[i][:], 0.0)
        nc.vector.memset(st32s[i][:, 64:65], 1.0 / dk)
        nc.scalar.copy(stcds[i][:], st32s[i][:])

    assert nseg % 2 == 0
    for T in range(nseg // 2):
        # superseg of 2 segments: projections at N=512, halving matmul+ldweights count
        xt_sb = xt_p.tile([128, 8, 2, SEG], CDT, tag="xt")
        nc.sync.dma_start(xt_sb[:], xt_d.ap()[2 * T : 2 * T + 2].rearrange("g p c s -> p c g s"))

        # ---- projections: qT,kT in [dk(2 heads), pack, l=512]; v natural ----
        qt = qk_p.tile([128, 4, 512], CDT, tag="qt")
        kt = qk_p.tile([128, 4, 512], CDT, tag="kt")
        vE = qk_p.tile([128, 4, HL, 65], CDT, tag="vE")
        for w_sb, dst in ((wq_sb, qt), (wk_sb, kt)):
            for pkk in range(4):
                prj = pp.tile([128, 512], F32, tag="ps")
                for kc in range(8):
                    nc.tensor.matmul(
                        prj[:],
                        w_sb[:, kc, pkk * 128 : (pkk + 1) * 128],
                        xt_sb[:, kc, :, :],
                        start=(kc == 0),
                        stop=(kc == 7),
                    )
                nc.vector.tensor_copy(dst[:, pkk, :], prj[:])
        for c in range(4):
            prj = pp.tile([128, 512], F32, tag="ps")
            for kc in range(8):
                nc.tensor.matmul(
                    prj[:],
                    xt_sb[:, kc, c // 2, (c % 2) * 128 : (c % 2) * 128 + 128],
                    wv_sb[:, kc, :],
                    start=(kc == 0),
                    stop=(kc == 7),
                )
            nc.vector.tensor_copy(vE[:, c, :, 0:64], prj[:].rearrange("p (h j) -> p h j", h=HL))
        nc.vector.memset(vE[:, :, :, 64:65], 1.0)

        # ---- elu(x)+1 = exp(min(x,0)) + max(x,0) ----
        sq = qk_p.tile([128, 4, 512], CDT, tag="sq")
        sk = qk_p.tile([128, 4, 512], F32, tag="sk")
        for src, dst in ((qt, sq), (kt, sk)):
            m0 = qk_p.tile([128, 4, 512], CDT, tag="m0")
            ex = qk_p.tile([128, 4, 512], CDT, tag="ex")
            nc.vector.tensor_scalar_min(m0[:], src[:], 0.0)
            nc.scalar.activation(ex[:], m0[:], AF.Exp)
            nc.vector.scalar_tensor_tensor(dst[:], src[:], 0.0, ex[:], op0=OP.max, op1=OP.add)

        for s in range(2):
            t = 2 * T + s
            so = s * SEG
            attT = at_p.tile([64, HL, SEG], CDT, tag="attT")

            for h in range(HL):
                hp, pk = h % 2, h // 2
                hs = hp * 64
                qTh = qt[hs : hs + 64, pk, so : so + SEG]
                sqh = sq[hs : hs + 64, pk, so : so + SEG]

                # scoresT[m, l] (2 m-chunks in one bank)
                scT = pa.tile([128, 2, SEG], F32, tag="ps")
                for mc in range(2):
                    nc.tensor.matmul(
                        scT[:, mc, :],
                        kt[hs : hs + 64, pk, so + mc * 128 : so + (mc + 1) * 128],
                        qTh,
                        start=True,
                        stop=True,
                    )
                # P^T = exp(scores/8) (no max subtraction; scores are O(1))
                PT = pt_p.tile([128, 2, SEG], CDT, tag="PT")
                nc.scalar.activation(PT[:], scT[:], AF.Exp, scale=0.125)

                # zBt: z broadcast along free, at this head's partitions
                zBt = sm_p.tile([128, 128], CDT, tag="zBt")
                nc.vector.tensor_scalar_mul(
                    zBt[hs : hs + 64, :], ones128c[hs : hs + 64, :], st32s[pk][hs : hs + 64, 64:65]
                )

                # U: dpaT_raw in [0:64, 0:256]; att_memT_raw in [0:64, 256:512]
                UU = pa.tile([128, 512], F32, tag="ps")
                for mc in range(2):
                    nc.tensor.matmul(
                        UU[0:64, 0:256], vE[:, 2 * s + mc, h, 0:64], PT[:, mc, :], start=(mc == 0), stop=(mc == 1)
                    )
                nc.tensor.matmul(UU[0:64, 256:512], stcds[pk][hs : hs + 64, 0:64], sqh, start=True, stop=True)

                # BB: broadcast denominators: sum_m P in [0:64, 0:256],
                # sigma_q @ z in [0:64, 256:512]
                BB = pa.tile([128, 512], F32, tag="ps")
                for mc in range(2):
                    nc.tensor.matmul(BB[0:64, 0:256], onesB[:, 0:64], PT[:, mc, :], start=(mc == 0), stop=(mc == 1))
                nc.tensor.matmul(BB[0:64, 256:512], zBt[hs : hs + 64, 0:64], sqh, start=True, stop=True)

                # reciprocal of both denominators: one fast custom-DVE op
                rb2 = sm_p.tile([64, 512], F32, tag="rb2")
                nc.vector.reciprocal_approx_fast(rb2[:], BB[0:64, 0:512])

                # combine: att = bsig * att_mem / zden + (1-bsig) * dpa / sden
                bn = sm_p.tile([64, SEG], F32, tag="bn")
                t2 = sm_p.tile([64, SEG], F32, tag="t2")
                nc.vector.scalar_tensor_tensor(
                    bn[:], UU[0:64, 256:512], bsig_sb[:, h : h + 1], rb2[:, 256:512], op0=OP.mult, op1=OP.mult
                )
                nc.vector.scalar_tensor_tensor(
                    t2[:], UU[0:64, 0:256], bsig1m_sb[:, h : h + 1], rb2[:, 0:256], op0=OP.mult, op1=OP.mult
                )
                nc.gpsimd.tensor_add(attT[:, h, :], bn[:], t2[:])

                # state update: mem += sigma_k^T v ; z += sum_l sigma_k
                trd = pa.tile([128, 256], F32, tag="ps")
                for mc in range(2):
                    nc.tensor.transpose(
                        trd[:, mc * 64 : (mc + 1) * 64],
                        sk[hs : hs + 64, pk, so + mc * 128 : so + (mc + 1) * 128],
                        ident[hs : hs + 64, 0:64],
                    )
                skn = sm_p.tile([128, 2, 64], CDT, tag="skn")
                for mc in range(2):
                    nc.scalar.copy(skn[:, mc, :], trd[:, mc * 64 : (mc + 1) * 64])
                for mc in range(2):
                    nc.tensor.matmul(
                        trd[hs : hs + 64, 128:193], skn[:, mc, :], vE[:, 2 * s + mc, h, :], start=(mc == 0), stop=(mc == 1)
                    )
                nc.vector.tensor_add(st32s[pk][hs : hs + 64, :], st32s[pk][hs : hs + 64, :], trd[hs : hs + 64, 128:193])
                nc.scalar.copy(stcds[pk][hs : hs + 64, :], st32s[pk][hs : hs + 64, :])

            # ---- output projection: out_rows = att_view @ Wout ----
            o_sb = ob_p.tile([128, 1024], F32, tag="o")
            aT = attT[:].rearrange("p h (lh lm) -> p h lh lm", lm=16)
            for nh in range(2):
                pot = po.tile([128, 512], F32, tag="ps")
                for lm in range(16):
                    rhs = wout_sb[:, lm, nh * 512 : (nh + 1) * 512]
                    nc.tensor.matmul(pot[:], aT[:, :, :, lm], rhs, start=(lm == 0), stop=(lm == 15))
                nc.scalar.copy(o_sb[:, nh * 512 : (nh + 1) * 512], pot[:])
            nc.sync.dma_start(out_d.ap()[t * 128 : (t + 1) * 128, :], o_sb[:])


def build_program(nseg=NSEG):
    nc = bacc.Bacc("TRN2", target_bir_lowering=False, debug=False, num_devices=NCORES)
    xt_d = nc.dram_tensor("xt", [nseg, 128, 8, SEG], CDT, kind="ExternalInput")
    wq_d = nc.dram_tensor("wq", [128, 8, 512], CDT, kind="ExternalInput")
    wk_d = nc.dram_tensor("wk", [128, 8, 512], CDT, kind="ExternalInput")
    wv_d = nc.dram_tensor("wv", [128, 8, 512], CDT, kind="ExternalInput")
    wout_d = nc.dram_tensor("wout", [64, 16, 1024], CDT, kind="ExternalInput")
    bsig_d = nc.dram_tensor("bsig", [64, HL], F32, kind="ExternalInput")
    bsig1m_d = nc.dram_tensor("bsig1m", [64, HL], F32, kind="ExternalInput")
    id64_d = nc.dram_tensor("id64", [128, 64], F32, kind="ExternalInput")
    out_d = nc.dram_tensor("out", [nseg * 128, 1024], F32, kind="ExternalOutput")
    with tile.TileContext(nc) as tc:
        with ExitStack() as ctx:
            _emit(ctx, tc, nseg, xt_d, wq_d, wk_d, wv_d, wout_d, bsig_d, bsig1m_d, id64_d, out_d)
    nc.compile()
    return nc


def shard_inputs(x, Wq, Wk, Wv, Wout, betas, nseg=NSEG):
    x = np.asarray(x, np.float32)
    Wq = np.asarray(Wq, np.float32)
    Wk = np.asarray(Wk, np.float32)
    Wv = np.asarray(Wv, np.float32)
    Wout = np.asarray(Wout, np.float32)
    betas = np.asarray(betas, np.float32)
    sig = 1.0 / (1.0 + np.exp(-betas[0, :, 0, :]))  # [H, dv]

    wout_t = np.ascontiguousarray(Wout.reshape(16, 64, 1024).transpose(1, 0, 2)).astype(NPDT)
    id64 = np.tile(np.eye(64, dtype=np.float32), (2, 1))
    in_maps = []
    for c in range(NCORES):
        b, hg = c // 2, c % 2
        hb = hg * HL
        xt = x[b].T.reshape(8, 128, S // SEG, SEG).transpose(2, 1, 0, 3)[:nseg]
        m = {
            "xt": np.ascontiguousarray(xt).astype(NPDT),
            "wq": np.ascontiguousarray(Wq[:, hb * 64 : (hb + HL) * 64].reshape(8, 128, 512).transpose(1, 0, 2)).astype(NPDT),
            "wk": np.ascontiguousarray(Wk[:, hb * 64 : (hb + HL) * 64].reshape(8, 128, 512).transpose(1, 0, 2)).astype(NPDT),
            "wv": np.ascontiguousarray(Wv[:, hb * 64 : (hb + HL) * 64].reshape(8, 128, 512).transpose(1, 0, 2)).astype(NPDT),
            "wout": wout_t,
            "bsig": np.ascontiguousarray(sig[hb : hb + HL].T),
            "bsig1m": np.ascontiguousarray((1.0 - sig)[hb : hb + HL].T),
            "id64": id64,
        }
        in_maps.append(m)
    return in_maps


def assemble_output(results, nseg=NSEG):
    out = np.empty((B, nseg * SEG, D), np.float32)
    o5 = out.reshape(B, nseg, 2, 128, D)
    for c in range(NCORES):
        b, hg = c // 2, c % 2
        o5[b, :, hg] = results[c]["out"].reshape(nseg, 128, D)
    return out


_COMPILED = {}


def _get_program(nseg=NSEG):
    if nseg not in _COMPILED:
        _COMPILED[nseg] = build_program(nseg)
    return _COMPILED[nseg]


def run(x, Wq, Wk, Wv, Wout, betas, nseg=NSEG, trace=False):
    nc = _get_program(nseg)
    in_maps = shard_inputs(x, Wq, Wk, Wv, Wout, betas, nseg)
    res = run_bass_kernel_spmd(nc, in_maps, list(range(NCORES)), trace=trace)
    return assemble_output(res.results, nseg), res.exec_time_ns


def kernel(x, Wq, Wk, Wv, Wout, betas):
    out, _ = run(x, Wq, Wk, Wv, Wout, betas, nseg=NSEG, trace=False)
    return out


# revision 32
# speedup vs baseline: 1.7331x; 1.1570x over previous
"""Trainium2 Bass kernel for CompressiveMemory (Infini-attention style).

Sharding: 8 cores = 4 batch x 2 head-groups (8 heads each). The reference's
`att.reshape(B, SEG, H*dv)` is a raw view of a (B,H,SEG,dv) tensor, so each
block of 16 output rows depends on exactly one head: head-sharding needs no
cross-core reduction, only row scattering (done on host).

Per-core kernel (all layouts chosen so matmul contractions land on the
partition dim, avoiding transposes except 2 tiny ones per head-segment):
  - host passes x[b] pre-transposed/tiled as xt[t, p, c, s]
  - per segment: project qT,kT (dk-on-partitions) and v (natural), elu+1
  - attention computed transposed: scoresT = kT.T-contracted, exp without
    max-subtraction (scores are O(1): q,k ~ N(0,1), scaled by 1/8), softmax
    denominator via ones-matmul partition reduction
  - memory state [dk, dv+1] per head (fp32 master + compute-dtype shadow)
  - output projection uses the raw-view structure: 16 rank-64 PSUM-accumulated
    matmuls whose lhsT operands are pure AP slices of the transposed att tile
"""

import os
import sys

for _p in ("/opt/trn_rl_repo",):
    if _p not in sys.path and os.path.isdir(_p):
        sys.path.insert(0, _p)

from contextlib import ExitStack

import ml_dtypes
import numpy as np

import concourse.bass as bass
import concourse.tile as tile
from concourse import bacc, mybir
from concourse.bass_utils import run_bass_kernel_spmd

AF = mybir.ActivationFunctionType
OP = mybir.AluOpType
F32 = mybir.dt.float32

B, S, D = 4, 8192, 1024
H, dk, dv, SEG = 16, 64, 64, 256
HL = 8  # heads per core
NCORES = 8

NSEG = int(os.environ.get("BASS_NSEG", S // SEG))
USE_BF16 = os.environ.get("BASS_CDT", "bf16") == "bf16"
CDT = mybir.dt.bfloat16 if USE_BF16 else F32
NPDT = ml_dtypes.bfloat16 if USE_BF16 else np.float32


def _emit(ctx, tc, nseg, xt_d, wq_d, wk_d, wv_d, wout_d, bsig_d, bsig1m_d, id64_d, out_d):
    nc = tc.nc

    consts = ctx.enter_context(tc.tile_pool(name="consts", bufs=1))
    state_p = ctx.enter_context(tc.tile_pool(name="state", bufs=1))
    xt_p = ctx.enter_context(tc.tile_pool(name="xtp", bufs=2))
    qk_p = ctx.enter_context(tc.tile_pool(name="qk", bufs=2))
    pt_p = ctx.enter_context(tc.tile_pool(name="ptp", bufs=4))
    at_p = ctx.enter_context(tc.tile_pool(name="atp", bufs=2))
    sm_p = ctx.enter_context(tc.tile_pool(name="smp", bufs=4))
    ob_p = ctx.enter_context(tc.tile_pool(name="obp", bufs=2))
    ps = ctx.enter_context(tc.tile_pool(name="ps", bufs=8, space="PSUM"))
    pp = pa = po = ps

    wq_sb = consts.tile([128, 8, 512], CDT, tag="wq")
    wk_sb = consts.tile([128, 8, 512], CDT, tag="wk")
    wv_sb = consts.tile([128, 8, 512], CDT, tag="wv")
    wout_sb = consts.tile([64, 16, 1024], CDT, tag="wout")
    nc.sync.dma_start(wq_sb[:], wq_d.ap())
    nc.sync.dma_start(wk_sb[:], wk_d.ap())
    nc.sync.dma_start(wv_sb[:], wv_d.ap())
    nc.sync.dma_start(wout_sb[:], wout_d.ap())
    bsig_sb = consts.tile([64, HL], F32, tag="bsig")
    bsig1m_sb = consts.tile([64, HL], F32, tag="bsig1m")
    nc.sync.dma_start(bsig_sb[:], bsig_d.ap())
    nc.sync.dma_start(bsig1m_sb[:], bsig1m_d.ap())
    ident = consts.tile([128, 64], F32, tag="ident")
    nc.sync.dma_start(ident[:], id64_d.ap())
    onesB = consts.tile([128, 128], CDT, tag="onesB")
    nc.vector.memset(onesB[:], 1.0)
    ones128c = consts.tile([128, 128], CDT, tag="ones128c")
    nc.vector.memset(ones128c[:], 1.0)

    # per-head memory state, 2 heads packed on partitions; one tile per pack
    # (separate tiles so one head-pair's update never serializes another's reads)
    st32s = [state_p.tile([128, 65], F32, tag=f"st32_{i}", name=f"st32_{i}") for i in range(HL // 2)]
    stcds = [state_p.tile([128, 65], CDT, tag=f"stcd_{i}", name=f"stcd_{i}") for i in range(HL // 2)]
    for i in range(HL // 2):
        nc.vector.memset(st32s[i][:], 0.0)
        nc.vector.memset(st32s[i][:, 64:65], 1.0 / dk)
        nc.scalar.copy(stcds[i][:], st32s[i][:])

    assert nseg % 2 == 0
    for T in range(nseg // 2):
        # superseg of 2 segments: projections at N=512, halving matmul+ldweights count
        xt_sb = xt_p.tile([128, 8, 2, SEG], CDT, tag="xt")
        nc.sync.dma_start(xt_sb[:], xt_d.ap()[2 * T : 2 * T + 2].rearrange("g p c s -> p c g s"))

        # ---- projections: qT,kT in [dk(2 heads), pack, l=512]; v natural ----
        qt = qk_p.tile([128, 4, 512], CDT, tag="qt")
        kt = qk_p.tile([128, 4, 512], CDT, tag="kt")
        vE = qk_p.tile([128, 4, HL, 65], CDT, tag="vE")
        for w_sb, dst in ((wq_sb, qt), (wk_sb, kt)):
            for pkk in range(4):
                prj = pp.tile([128, 512], F32, tag="ps")
                for kc in range(8):
                    nc.tensor.matmul(
                        prj[:],
                        w_sb[:, kc, pkk * 128 : (pkk + 1) * 128],
                        xt_sb[:, kc, :, :],
                        start=(kc == 0),
                        stop=(kc == 7),
                    )
                nc.vector.tensor_copy(dst[:, pkk, :], prj[:])
        for c in range(4):
            prj = pp.tile([128, 512], F32, tag="ps")
            for kc in range(8):
                nc.tensor.matmul(
                    prj[:],
                    xt_sb[:, kc, c // 2, (c % 2) * 128 : (c % 2) * 128 + 128],
                    wv_sb[:, kc, :],
                    start=(kc == 0),
                    stop=(kc == 7),
                )
            nc.vector.tensor_copy(vE[:, c, :, 0:64], prj[:].rearrange("p (h j) -> p h j", h=HL))
        nc.vector.memset(vE[:, :, :, 64:65], 1.0)

        # ---- elu(x)+1 = exp(min(x,0)) + max(x,0) ----
        sq = qk_p.tile([128, 4, 512], CDT, tag="sq")
        sk = qk_p.tile([128, 4, 512], F32, tag="sk")
        for src, dst in ((qt, sq), (kt, sk)):
            m0 = qk_p.tile([128, 4, 512], CDT, tag="m0")
            ex = qk_p.tile([128, 4, 512], CDT, tag="ex")
            nc.vector.tensor_scalar_min(m0[:], src[:], 0.0)
            nc.scalar.activation(ex[:], m0[:], AF.Exp)
            nc.vector.scalar_tensor_tensor(dst[:], src[:], 0.0, ex[:], op0=OP.max, op1=OP.add)

        for s in range(2):
            t = 2 * T + s
            so = s * SEG
            attT = at_p.tile([64, HL, SEG], CDT, tag="attT")

            def stage_a(h):
                hp, pk = h % 2, h // 2
                hs = hp * 64
                qTh = qt[hs : hs + 64, pk, so : so + SEG]
                sqh = sq[hs : hs + 64, pk, so : so + SEG]

                # zBt: z broadcast along free, at this head's partitions
                zBt = sm_p.tile([128, 128], CDT, tag="zBt")
                nc.vector.tensor_scalar_mul(
                    zBt[hs : hs + 64, :], ones128c[hs : hs + 64, :], st32s[pk][hs : hs + 64, 64:65]
                )

                # scoresT[m, l] (2 m-chunks in one bank)
                scT = pa.tile([128, 2, SEG], F32, tag="ps")
                for mc in range(2):
                    nc.tensor.matmul(
                        scT[:, mc, :],
                        kt[hs : hs + 64, pk, so + mc * 128 : so + (mc + 1) * 128],
                        qTh,
                        start=True,
                        stop=True,
                    )
                # P^T = exp(scores/8) (no max subtraction; scores are O(1))
                PT = pt_p.tile([128, 2, SEG], CDT, tag="PT")
                nc.scalar.activation(PT[:], scT[:], AF.Exp, scale=0.125)

                # U: att_memT_raw in [0:64, 256:512]; (dpaT added in stage b)
                UU = pa.tile([128, 512], F32, tag="ps")
                nc.tensor.matmul(UU[0:64, 256:512], stcds[pk][hs : hs + 64, 0:64], sqh, start=True, stop=True)
                # BB: sigma_q @ z broadcast in [0:64, 256:512]; (sum_m P in stage b)
                BB = pa.tile([128, 512], F32, tag="ps")
                nc.tensor.matmul(BB[0:64, 256:512], zBt[hs : hs + 64, 0:64], sqh, start=True, stop=True)

                # sigma_k transposes (PE) + copies to SBUF (ACT)
                trd = pa.tile([128, 256], F32, tag="ps")
                for mc in range(2):
                    nc.tensor.transpose(
                        trd[:, mc * 64 : (mc + 1) * 64],
                        sk[hs : hs + 64, pk, so + mc * 128 : so + (mc + 1) * 128],
                        ident[hs : hs + 64, 0:64],
                    )
                skn = sm_p.tile([128, 2, 64], CDT, tag="skn")
                for mc in range(2):
                    nc.scalar.copy(skn[:, mc, :], trd[:, mc * 64 : (mc + 1) * 64])
                return (h, PT, UU, BB, trd, skn)

            def stage_b(st):
                h, PT, UU, BB, trd, skn = st
                hp, pk = h % 2, h // 2
                hs = hp * 64
                for mc in range(2):
                    nc.tensor.matmul(
                        UU[0:64, 0:256], vE[:, 2 * s + mc, h, 0:64], PT[:, mc, :], start=(mc == 0), stop=(mc == 1)
                    )
                for mc in range(2):
                    nc.tensor.matmul(BB[0:64, 0:256], onesB[:, 0:64], PT[:, mc, :], start=(mc == 0), stop=(mc == 1))
                for mc in range(2):
                    nc.tensor.matmul(
                        trd[hs : hs + 64, 128:193], skn[:, mc, :], vE[:, 2 * s + mc, h, :], start=(mc == 0), stop=(mc == 1)
                    )

                # reciprocal of both denominators: one fast custom-DVE op
                rb2 = sm_p.tile([64, 512], F32, tag="rb2")
                nc.vector.reciprocal_approx_fast(rb2[:], BB[0:64, 0:512])

                # combine: att = bsig * att_mem / zden + (1-bsig) * dpa / sden
                bn = sm_p.tile([64, SEG], F32, tag="bn")
                t2 = sm_p.tile([64, SEG], F32, tag="t2")
                nc.vector.scalar_tensor_tensor(
                    bn[:], UU[0:64, 256:512], bsig_sb[:, h : h + 1], rb2[:, 256:512], op0=OP.mult, op1=OP.mult
                )
                nc.vector.scalar_tensor_tensor(
                    t2[:], UU[0:64, 0:256], bsig1m_sb[:, h : h + 1], rb2[:, 0:256], op0=OP.mult, op1=OP.mult
                )
                nc.gpsimd.tensor_add(attT[:, h, :], bn[:], t2[:])

                # state update: mem += sigma_k^T v ; z += sum_l sigma_k
                nc.vector.tensor_add(st32s[pk][hs : hs + 64, :], st32s[pk][hs : hs + 64, :], trd[hs : hs + 64, 128:193])
                nc.scalar.copy(stcds[pk][hs : hs + 64, :], st32s[pk][hs : hs + 64, :])

            # software-pipelined head loop; order alternates packs so stage_a(h)
            # never reads a state tile stage_b(prev) is writing
            horder = [0, 2, 4, 6, 1, 3, 5, 7]
            pending = stage_a(horder[0])
            for h in horder[1:]:
                nxt = stage_a(h)
                stage_b(pending)
                pending = nxt
            stage_b(pending)

            # ---- output projection: out_rows = att_view @ Wout ----
            o_sb = ob_p.tile([128, 1024], F32, tag="o")
            aT = attT[:].rearrange("p h (lh lm) -> p h lh lm", lm=16)
            for nh in range(2):
                pot = po.tile([128, 512], F32, tag="ps")
                for lm in range(16):
                    rhs = wout_sb[:, lm, nh * 512 : (nh + 1) * 512]
                    nc.tensor.matmul(pot[:], aT[:, :, :, lm], rhs, start=(lm == 0), stop=(lm == 15))
                nc.scalar.copy(o_sb[:, nh * 512 : (nh + 1) * 512], pot[:])
            nc.sync.dma_start(out_d.ap()[t * 128 : (t + 1) * 128, :], o_sb[:])


def build_program(nseg=NSEG):
    nc = bacc.Bacc("TRN2", target_bir_lowering=False, debug=False, num_devices=NCORES)
    xt_d = nc.dram_tensor("xt", [nseg, 128, 8, SEG], CDT, kind="ExternalInput")
    wq_d = nc.dram_tensor("wq", [128, 8, 512], CDT, kind="ExternalInput")
    wk_d = nc.dram_tensor("wk", [128, 8, 512], CDT, kind="ExternalInput")
    wv_d = nc.dram_tensor("wv", [128, 8, 512], CDT, kind="ExternalInput")
    wout_d = nc.dram_tensor("wout", [64, 16, 1024], CDT, kind="ExternalInput")
    bsig_d = nc.dram_tensor("bsig", [64, HL], F32, kind="ExternalInput")
    bsig1m_d = nc.dram_tensor("bsig1m", [64, HL], F32, kind="ExternalInput")
    id64_d = nc.dram_tensor("id64", [128, 64], F32, kind="ExternalInput")
    out_d = nc.dram_tensor("out", [nseg * 128, 1024], F32, kind="ExternalOutput")
    with tile.TileContext(nc) as tc:
        with ExitStack() as ctx:
            _emit(ctx, tc, nseg, xt_d, wq_d, wk_d, wv_d, wout_d, bsig_d, bsig1m_d, id64_d, out_d)
    nc.compile()
    return nc


def shard_inputs(x, Wq, Wk, Wv, Wout, betas, nseg=NSEG):
    x = np.asarray(x, np.float32)
    Wq = np.asarray(Wq, np.float32)
    Wk = np.asarray(Wk, np.float32)
    Wv = np.asarray(Wv, np.float32)
    Wout = np.asarray(Wout, np.float32)
    betas = np.asarray(betas, np.float32)
    sig = 1.0 / (1.0 + np.exp(-betas[0, :, 0, :]))  # [H, dv]

    wout_t = np.ascontiguousarray(Wout.reshape(16, 64, 1024).transpose(1, 0, 2)).astype(NPDT)
    id64 = np.tile(np.eye(64, dtype=np.float32), (2, 1))
    in_maps = []
    for c in range(NCORES):
        b, hg = c // 2, c % 2
        hb = hg * HL
        xt = x[b].T.reshape(8, 128, S // SEG, SEG).transpose(2, 1, 0, 3)[:nseg]
        m = {
            "xt": np.ascontiguousarray(xt).astype(NPDT),
            "wq": np.ascontiguousarray(Wq[:, hb * 64 : (hb + HL) * 64].reshape(8, 128, 512).transpose(1, 0, 2)).astype(NPDT),
            "wk": np.ascontiguousarray(Wk[:, hb * 64 : (hb + HL) * 64].reshape(8, 128, 512).transpose(1, 0, 2)).astype(NPDT),
            "wv": np.ascontiguousarray(Wv[:, hb * 64 : (hb + HL) * 64].reshape(8, 128, 512).transpose(1, 0, 2)).astype(NPDT),
            "wout": wout_t,
            "bsig": np.ascontiguousarray(sig[hb : hb + HL].T),
            "bsig1m": np.ascontiguousarray((1.0 - sig)[hb : hb + HL].T),
            "id64": id64,
        }
        in_maps.append(m)
    return in_maps


def assemble_output(results, nseg=NSEG):
    out = np.empty((B, nseg * SEG, D), np.float32)
    o5 = out.reshape(B, nseg, 2, 128, D)
    for c in range(NCORES):
        b, hg = c // 2, c % 2
        o5[b, :, hg] = results[c]["out"].reshape(nseg, 128, D)
    return out


_COMPILED = {}


def _get_program(nseg=NSEG):
    if nseg not in _COMPILED:
        _COMPILED[nseg] = build_program(nseg)
    return _COMPILED[nseg]


def run(x, Wq, Wk, Wv, Wout, betas, nseg=NSEG, trace=False):
    nc = _get_program(nseg)
    in_maps = shard_inputs(x, Wq, Wk, Wv, Wout, betas, nseg)
    res = run_bass_kernel_spmd(nc, in_maps, list(range(NCORES)), trace=trace)
    return assemble_output(res.results, nseg), res.exec_time_ns


def kernel(x, Wq, Wk, Wv, Wout, betas):
    out, _ = run(x, Wq, Wk, Wv, Wout, betas, nseg=NSEG, trace=False)
    return out


# revision 34
# speedup vs baseline: 1.9300x; 1.1136x over previous
"""Trainium2 Bass kernel for CompressiveMemory (Infini-attention style).

Sharding: 8 cores = 4 batch x 2 head-groups (8 heads each). The reference's
`att.reshape(B, SEG, H*dv)` is a raw view of a (B,H,SEG,dv) tensor, so each
block of 16 output rows depends on exactly one head: head-sharding needs no
cross-core reduction, only row scattering (done on host).

Per-core kernel (all layouts chosen so matmul contractions land on the
partition dim, avoiding transposes except 2 tiny ones per head-segment):
  - host passes x[b] pre-transposed/tiled as xt[t, p, c, s]
  - per segment: project qT,kT (dk-on-partitions) and v (natural), elu+1
  - attention computed transposed: scoresT = kT.T-contracted, exp without
    max-subtraction (scores are O(1): q,k ~ N(0,1), scaled by 1/8), softmax
    denominator via ones-matmul partition reduction
  - memory state [dk, dv+1] per head (fp32 master + compute-dtype shadow)
  - output projection uses the raw-view structure: 16 rank-64 PSUM-accumulated
    matmuls whose lhsT operands are pure AP slices of the transposed att tile
"""

import os
import sys

for _p in ("/opt/trn_rl_repo",):
    if _p not in sys.path and os.path.isdir(_p):
        sys.path.insert(0, _p)

from contextlib import ExitStack

import ml_dtypes
import numpy as np

import concourse.bass as bass
import concourse.tile as tile
from concourse import bacc, mybir
from concourse.bass_utils import run_bass_kernel_spmd

AF = mybir.ActivationFunctionType
OP = mybir.AluOpType
F32 = mybir.dt.float32

B, S, D = 4, 8192, 1024
H, dk, dv, SEG = 16, 64, 64, 256
HL = 8  # heads per core
NCORES = 8

NSEG = int(os.environ.get("BASS_NSEG", S // SEG))
USE_BF16 = os.environ.get("BASS_CDT", "bf16") == "bf16"
CDT = mybir.dt.bfloat16 if USE_BF16 else F32
NPDT = ml_dtypes.bfloat16 if USE_BF16 else np.float32


def _emit(ctx, tc, nseg, xt_d, wq_d, wk_d, wv_d, wout_d, bsig_d, bsig1m_d, id64_d, out_d):
    nc = tc.nc

    consts = ctx.enter_context(tc.tile_pool(name="consts", bufs=1))
    state_p = ctx.enter_context(tc.tile_pool(name="state", bufs=1))
    xt_p = ctx.enter_context(tc.tile_pool(name="xtp", bufs=2))
    qk_p = ctx.enter_context(tc.tile_pool(name="qk", bufs=2))
    pt_p = ctx.enter_context(tc.tile_pool(name="ptp", bufs=4))
    at_p = ctx.enter_context(tc.tile_pool(name="atp", bufs=2))
    sm_p = ctx.enter_context(tc.tile_pool(name="smp", bufs=4))
    ob_p = ctx.enter_context(tc.tile_pool(name="obp", bufs=2))
    ps = ctx.enter_context(tc.tile_pool(name="ps", bufs=8, space="PSUM"))
    pp = pa = po = ps

    wq_sb = consts.tile([128, 8, 512], CDT, tag="wq")
    wk_sb = consts.tile([128, 8, 512], CDT, tag="wk")
    wv_sb = consts.tile([128, 8, 512], CDT, tag="wv")
    wout_sb = consts.tile([64, 16, 1024], CDT, tag="wout")
    nc.sync.dma_start(wq_sb[:], wq_d.ap())
    nc.sync.dma_start(wk_sb[:], wk_d.ap())
    nc.sync.dma_start(wv_sb[:], wv_d.ap())
    nc.sync.dma_start(wout_sb[:], wout_d.ap())
    bsig_sb = consts.tile([64, HL], F32, tag="bsig")
    bsig1m_sb = consts.tile([64, HL], F32, tag="bsig1m")
    nc.sync.dma_start(bsig_sb[:], bsig_d.ap())
    nc.sync.dma_start(bsig1m_sb[:], bsig1m_d.ap())
    ident = consts.tile([128, 64], F32, tag="ident")
    nc.sync.dma_start(ident[:], id64_d.ap())
    onesB = consts.tile([128, 128], CDT, tag="onesB")
    nc.vector.memset(onesB[:], 1.0)
    ones128c = consts.tile([128, 128], CDT, tag="ones128c")
    nc.vector.memset(ones128c[:], 1.0)

    # per-head memory state, 2 heads packed on partitions; one tile per pack
    # (separate tiles so one head-pair's update never serializes another's reads)
    st32s = [state_p.tile([128, 65], F32, tag=f"st32_{i}", name=f"st32_{i}") for i in range(HL // 2)]
    stcds = [state_p.tile([128, 65], CDT, tag=f"stcd_{i}", name=f"stcd_{i}") for i in range(HL // 2)]
    for i in range(HL // 2):
        nc.vector.memset(st32s[i][:], 0.0)
        nc.vector.memset(st32s[i][:, 64:65], 1.0 / dk)
        nc.scalar.copy(stcds[i][:], st32s[i][:])

    assert nseg % 2 == 0
    prev_out = None
    for T in range(nseg // 2):
        # superseg of 2 segments: projections at N=512, halving matmul+ldweights count
        xt_sb = xt_p.tile([128, 8, 2, SEG], CDT, tag="xt")
        nc.sync.dma_start(xt_sb[:], xt_d.ap()[2 * T : 2 * T + 2].rearrange("g p c s -> p c g s"))

        # ---- projections: qT,kT in [dk(2 heads), pack, l=512]; v natural ----
        qt = qk_p.tile([128, 4, 512], CDT, tag="qt")
        kt = qk_p.tile([128, 4, 512], CDT, tag="kt")
        vE = qk_p.tile([128, 4, HL, 65], CDT, tag="vE")
        for w_sb, dst in ((wq_sb, qt), (wk_sb, kt)):
            for pkk in range(4):
                prj = pp.tile([128, 512], F32, tag="ps")
                for kc in range(8):
                    nc.tensor.matmul(
                        prj[:],
                        w_sb[:, kc, pkk * 128 : (pkk + 1) * 128],
                        xt_sb[:, kc, :, :],
                        start=(kc == 0),
                        stop=(kc == 7),
                    )
                nc.vector.tensor_copy(dst[:, pkk, :], prj[:])
        for c in range(4):
            prj = pp.tile([128, 512], F32, tag="ps")
            for kc in range(8):
                nc.tensor.matmul(
                    prj[:],
                    xt_sb[:, kc, c // 2, (c % 2) * 128 : (c % 2) * 128 + 128],
                    wv_sb[:, kc, :],
                    start=(kc == 0),
                    stop=(kc == 7),
                )
            nc.vector.tensor_copy(vE[:, c, :, 0:64], prj[:].rearrange("p (h j) -> p h j", h=HL))
        nc.vector.memset(vE[:, :, :, 64:65], 1.0)

        # ---- elu(x)+1 = exp(min(x,0)) + max(x,0) ----
        sq = qk_p.tile([128, 4, 512], CDT, tag="sq")
        sk = qk_p.tile([128, 4, 512], F32, tag="sk")
        for src, dst in ((qt, sq), (kt, sk)):
            m0 = qk_p.tile([128, 4, 512], CDT, tag="m0")
            ex = qk_p.tile([128, 4, 512], CDT, tag="ex")
            nc.vector.tensor_scalar_min(m0[:], src[:], 0.0)
            nc.scalar.activation(ex[:], m0[:], AF.Exp)
            nc.vector.scalar_tensor_tensor(dst[:], src[:], 0.0, ex[:], op0=OP.max, op1=OP.add)

        for s in range(2):
            t = 2 * T + s
            so = s * SEG
            attT = at_p.tile([64, HL, SEG], CDT, tag="attT")

            def stage_a(h):
                hp, pk = h % 2, h // 2
                hs = hp * 64
                qTh = qt[hs : hs + 64, pk, so : so + SEG]
                sqh = sq[hs : hs + 64, pk, so : so + SEG]

                # zBt: z broadcast along free, at this head's partitions
                zBt = sm_p.tile([128, 128], CDT, tag="zBt")
                nc.vector.tensor_scalar_mul(
                    zBt[hs : hs + 64, :], ones128c[hs : hs + 64, :], st32s[pk][hs : hs + 64, 64:65]
                )

                # scoresT[m, l] (2 m-chunks in one bank)
                scT = pa.tile([128, 2, SEG], F32, tag="ps")
                for mc in range(2):
                    nc.tensor.matmul(
                        scT[:, mc, :],
                        kt[hs : hs + 64, pk, so + mc * 128 : so + (mc + 1) * 128],
                        qTh,
                        start=True,
                        stop=True,
                    )
                # P^T = exp(scores/8) (no max subtraction; scores are O(1))
                PT = pt_p.tile([128, 2, SEG], CDT, tag="PT")
                nc.scalar.activation(PT[:], scT[:], AF.Exp, scale=0.125)

                # U: att_memT_raw in [0:64, 256:512]; (dpaT added in stage b)
                UU = pa.tile([128, 512], F32, tag="ps")
                nc.tensor.matmul(UU[0:64, 256:512], stcds[pk][hs : hs + 64, 0:64], sqh, start=True, stop=True)
                # BB: sigma_q @ z broadcast in [0:64, 256:512]; (sum_m P in stage b)
                BB = pa.tile([128, 512], F32, tag="ps")
                nc.tensor.matmul(BB[0:64, 256:512], zBt[hs : hs + 64, 0:64], sqh, start=True, stop=True)

                # sigma_k transposes (PE) + copies to SBUF (ACT)
                trd = pa.tile([128, 256], F32, tag="ps")
                for mc in range(2):
                    nc.tensor.transpose(
                        trd[:, mc * 64 : (mc + 1) * 64],
                        sk[hs : hs + 64, pk, so + mc * 128 : so + (mc + 1) * 128],
                        ident[hs : hs + 64, 0:64],
                    )
                skn = sm_p.tile([128, 2, 64], CDT, tag="skn")
                for mc in range(2):
                    nc.scalar.copy(skn[:, mc, :], trd[:, mc * 64 : (mc + 1) * 64])
                return (h, PT, UU, BB, trd, skn)

            def stage_b(st):
                h, PT, UU, BB, trd, skn = st
                hp, pk = h % 2, h // 2
                hs = hp * 64
                for mc in range(2):
                    nc.tensor.matmul(
                        UU[0:64, 0:256], vE[:, 2 * s + mc, h, 0:64], PT[:, mc, :], start=(mc == 0), stop=(mc == 1)
                    )
                for mc in range(2):
                    nc.tensor.matmul(BB[0:64, 0:256], onesB[:, 0:64], PT[:, mc, :], start=(mc == 0), stop=(mc == 1))
                for mc in range(2):
                    nc.tensor.matmul(
                        trd[hs : hs + 64, 128:193], skn[:, mc, :], vE[:, 2 * s + mc, h, :], start=(mc == 0), stop=(mc == 1)
                    )

                # reciprocal of both denominators: one fast custom-DVE op
                rb2 = sm_p.tile([64, 512], F32, tag="rb2")
                nc.vector.reciprocal_approx_fast(rb2[:], BB[0:64, 0:512])

                # combine: att = bsig * att_mem / zden + (1-bsig) * dpa / sden
                bn = sm_p.tile([64, SEG], F32, tag="bn")
                t2 = sm_p.tile([64, SEG], F32, tag="t2")
                nc.vector.scalar_tensor_tensor(
                    bn[:], UU[0:64, 256:512], bsig_sb[:, h : h + 1], rb2[:, 256:512], op0=OP.mult, op1=OP.mult
                )
                nc.vector.scalar_tensor_tensor(
                    t2[:], UU[0:64, 0:256], bsig1m_sb[:, h : h + 1], rb2[:, 0:256], op0=OP.mult, op1=OP.mult
                )
                nc.gpsimd.tensor_add(attT[:, h, :], bn[:], t2[:])

                # state update: mem += sigma_k^T v ; z += sum_l sigma_k
                nc.vector.tensor_add(st32s[pk][hs : hs + 64, :], st32s[pk][hs : hs + 64, :], trd[hs : hs + 64, 128:193])
                nc.scalar.copy(stcds[pk][hs : hs + 64, :], st32s[pk][hs : hs + 64, :])

            # software-pipelined head loop; order alternates packs so stage_a(h)
            # never reads a state tile stage_b(prev) is writing. The previous
            # segment's output projection is emitted after the first two
            # stage_a's so its PE work lands where the combine tail would
            # otherwise stall the tensor engine.
            horder = [0, 2, 4, 6, 1, 3, 5, 7]
            pending = stage_a(horder[0])
            nxt = stage_a(horder[1])
            if prev_out is not None:
                prev_out()
                prev_out = None
            stage_b(pending)
            pending = nxt
            for h in horder[2:]:
                nxt = stage_a(h)
                stage_b(pending)
                pending = nxt
            stage_b(pending)

            def make_outproj(attT, t):
                def emit():
                    # out_rows = att_view @ Wout (raw-view structure: 16 rank-64 updates)
                    o_sb = ob_p.tile([128, 1024], F32, tag="o")
                    aT = attT[:].rearrange("p h (lh lm) -> p h lh lm", lm=16)
                    for nh in range(2):
                        pot = po.tile([128, 512], F32, tag="ps")
                        for lm in range(16):
                            rhs = wout_sb[:, lm, nh * 512 : (nh + 1) * 512]
                            nc.tensor.matmul(pot[:], aT[:, :, :, lm], rhs, start=(lm == 0), stop=(lm == 15))
                        nc.scalar.copy(o_sb[:, nh * 512 : (nh + 1) * 512], pot[:])
                    nc.sync.dma_start(out_d.ap()[t * 128 : (t + 1) * 128, :], o_sb[:])

                return emit

            prev_out = make_outproj(attT, t)
    prev_out()


def build_program(nseg=NSEG):
    nc = bacc.Bacc("TRN2", target_bir_lowering=False, debug=False, num_devices=NCORES)
    xt_d = nc.dram_tensor("xt", [nseg, 128, 8, SEG], CDT, kind="ExternalInput")
    wq_d = nc.dram_tensor("wq", [128, 8, 512], CDT, kind="ExternalInput")
    wk_d = nc.dram_tensor("wk", [128, 8, 512], CDT, kind="ExternalInput")
    wv_d = nc.dram_tensor("wv", [128, 8, 512], CDT, kind="ExternalInput")
    wout_d = nc.dram_tensor("wout", [64, 16, 1024], CDT, kind="ExternalInput")
    bsig_d = nc.dram_tensor("bsig", [64, HL], F32, kind="ExternalInput")
    bsig1m_d = nc.dram_tensor("bsig1m", [64, HL], F32, kind="ExternalInput")
    id64_d = nc.dram_tensor("id64", [128, 64], F32, kind="ExternalInput")
    out_d = nc.dram_tensor("out", [nseg * 128, 1024], F32, kind="ExternalOutput")
    with tile.TileContext(nc) as tc:
        with ExitStack() as ctx:
            _emit(ctx, tc, nseg, xt_d, wq_d, wk_d, wv_d, wout_d, bsig_d, bsig1m_d, id64_d, out_d)
    nc.compile()
    return nc


def shard_inputs(x, Wq, Wk, Wv, Wout, betas, nseg=NSEG):
    x = np.asarray(x, np.float32)
    Wq = np.asarray(Wq, np.float32)
    Wk = np.asarray(Wk, np.float32)
    Wv = np.asarray(Wv, np.float32)
    Wout = np.asarray(Wout, np.float32)
    betas = np.asarray(betas, np.float32)
    sig = 1.0 / (1.0 + np.exp(-betas[0, :, 0, :]))  # [H, dv]

    wout_t = np.ascontiguousarray(Wout.reshape(16, 64, 1024).transpose(1, 0, 2)).astype(NPDT)
    id64 = np.tile(np.eye(64, dtype=np.float32), (2, 1))
    in_maps = []
    for c in range(NCORES):
        b, hg = c // 2, c % 2
        hb = hg * HL
        xt = x[b].T.reshape(8, 128, S // SEG, SEG).transpose(2, 1, 0, 3)[:nseg]
        m = {
            "xt": np.ascontiguousarray(xt).astype(NPDT),
            "wq": np.ascontiguousarray(Wq[:, hb * 64 : (hb + HL) * 64].reshape(8, 128, 512).transpose(1, 0, 2)).astype(NPDT),
            "wk": np.ascontiguousarray(Wk[:, hb * 64 : (hb + HL) * 64].reshape(8, 128, 512).transpose(1, 0, 2)).astype(NPDT),
            "wv": np.ascontiguousarray(Wv[:, hb * 64 : (hb + HL) * 64].reshape(8, 128, 512).transpose(1, 0, 2)).astype(NPDT),
            "wout": wout_t,
            "bsig": np.ascontiguousarray(sig[hb : hb + HL].T),
            "bsig1m": np.ascontiguousarray((1.0 - sig)[hb : hb + HL].T),
            "id64": id64,
        }
        in_maps.append(m)
    return in_maps


def assemble_output(results, nseg=NSEG):
    out = np.empty((B, nseg * SEG, D), np.float32)
    o5 = out.reshape(B, nseg, 2, 128, D)
    for c in range(NCORES):
        b, hg = c // 2, c % 2
        o5[b, :, hg] = results[c]["out"].reshape(nseg, 128, D)
    return out


_COMPILED = {}


def _get_program(nseg=NSEG):
    if nseg not in _COMPILED:
        _COMPILED[nseg] = build_program(nseg)
    return _COMPILED[nseg]


def run(x, Wq, Wk, Wv, Wout, betas, nseg=NSEG, trace=False):
    nc = _get_program(nseg)
    in_maps = shard_inputs(x, Wq, Wk, Wv, Wout, betas, nseg)
    res = run_bass_kernel_spmd(nc, in_maps, list(range(NCORES)), trace=trace)
    return assemble_output(res.results, nseg), res.exec_time_ns


def kernel(x, Wq, Wk, Wv, Wout, betas):
    out, _ = run(x, Wq, Wk, Wv, Wout, betas, nseg=NSEG, trace=False)
    return out


# revision 40
# speedup vs baseline: 2.0393x; 1.0566x over previous
"""Trainium2 Bass kernel for CompressiveMemory (Infini-attention style).

Sharding: 8 cores = 4 batch x 2 head-groups (8 heads each). The reference's
`att.reshape(B, SEG, H*dv)` is a raw view of a (B,H,SEG,dv) tensor, so each
block of 16 output rows depends on exactly one head: head-sharding needs no
cross-core reduction, only row scattering (done on host).

Per-core kernel (all layouts chosen so matmul contractions land on the
partition dim, avoiding transposes except 2 tiny ones per head-segment):
  - host passes x[b] pre-transposed/tiled as xt[t, p, c, s]
  - per segment: project qT,kT (dk-on-partitions) and v (natural), elu+1
  - attention computed transposed: scoresT = kT.T-contracted, exp without
    max-subtraction (scores are O(1): q,k ~ N(0,1), scaled by 1/8), softmax
    denominator via ones-matmul partition reduction
  - memory state [dk, dv+1] per head (fp32 master + compute-dtype shadow)
  - output projection uses the raw-view structure: 16 rank-64 PSUM-accumulated
    matmuls whose lhsT operands are pure AP slices of the transposed att tile
"""

import os
import sys

for _p in ("/opt/trn_rl_repo",):
    if _p not in sys.path and os.path.isdir(_p):
        sys.path.insert(0, _p)

from contextlib import ExitStack

import ml_dtypes
import numpy as np

import concourse.bass as bass
import concourse.tile as tile
from concourse import bacc, mybir
from concourse.bass_utils import run_bass_kernel_spmd

AF = mybir.ActivationFunctionType
OP = mybir.AluOpType
F32 = mybir.dt.float32

B, S, D = 4, 8192, 1024
H, dk, dv, SEG = 16, 64, 64, 256
HL = 8  # heads per core
NCORES = 8

NSEG = int(os.environ.get("BASS_NSEG", S // SEG))
USE_BF16 = os.environ.get("BASS_CDT", "bf16") == "bf16"
CDT = mybir.dt.bfloat16 if USE_BF16 else F32
NPDT = ml_dtypes.bfloat16 if USE_BF16 else np.float32


def _emit(ctx, tc, nseg, xt_d, wq_d, wk_d, wv_d, wout_d, bsig_d, bsig1m_d, id64_d, out_d):
    nc = tc.nc

    consts = ctx.enter_context(tc.tile_pool(name="consts", bufs=1))
    state_p = ctx.enter_context(tc.tile_pool(name="state", bufs=1))
    xt_p = ctx.enter_context(tc.tile_pool(name="xtp", bufs=2))
    qk_p = ctx.enter_context(tc.tile_pool(name="qk", bufs=2))
    pt_p = ctx.enter_context(tc.tile_pool(name="ptp", bufs=4))
    at_p = ctx.enter_context(tc.tile_pool(name="atp", bufs=2))
    sm_p = ctx.enter_context(tc.tile_pool(name="smp", bufs=4))
    ob_p = ctx.enter_context(tc.tile_pool(name="obp", bufs=2))
    ps = ctx.enter_context(tc.tile_pool(name="ps", bufs=8, space="PSUM"))
    pp = pa = po = ps

    wq_sb = consts.tile([128, 8, 512], CDT, tag="wq")
    wk_sb = consts.tile([128, 8, 512], CDT, tag="wk")
    wv_sb = consts.tile([128, 8, 512], CDT, tag="wv")
    wout_sb = consts.tile([128, 8, 1024], CDT, tag="wout")
    nc.sync.dma_start(wq_sb[:], wq_d.ap())
    nc.sync.dma_start(wk_sb[:], wk_d.ap())
    nc.sync.dma_start(wv_sb[:], wv_d.ap())
    nc.sync.dma_start(wout_sb[:], wout_d.ap())
    bsig_sb = consts.tile([64, HL], F32, tag="bsig")
    bsig1m_sb = consts.tile([64, HL], F32, tag="bsig1m")
    nc.sync.dma_start(bsig_sb[:], bsig_d.ap())
    nc.sync.dma_start(bsig1m_sb[:], bsig1m_d.ap())
    ident = consts.tile([128, 64], F32, tag="ident")
    nc.sync.dma_start(ident[:], id64_d.ap())
    onesB = consts.tile([128, 128], CDT, tag="onesB")
    nc.vector.memset(onesB[:], 1.0)
    ones128c = consts.tile([128, 128], CDT, tag="ones128c")
    nc.vector.memset(ones128c[:], 1.0)

    # per-head memory state, 2 heads packed on partitions; one tile per pack
    # (separate tiles so one head-pair's update never serializes another's reads)
    st32s = [state_p.tile([128, 65], F32, tag=f"st32_{i}", name=f"st32_{i}") for i in range(HL // 2)]
    stcds = [state_p.tile([128, 65], CDT, tag=f"stcd_{i}", name=f"stcd_{i}") for i in range(HL // 2)]
    for i in range(HL // 2):
        nc.vector.memset(st32s[i][:], 0.0)
        nc.vector.memset(st32s[i][:, 64:65], 1.0 / dk)
        nc.scalar.copy(stcds[i][:], st32s[i][:])

    assert nseg % 2 == 0
    prev_out = None
    for T in range(nseg // 2):
        # superseg of 2 segments: projections at N=512, halving matmul+ldweights count
        xt_sb = xt_p.tile([128, 8, 2, SEG], CDT, tag="xt")
        nc.sync.dma_start(xt_sb[:], xt_d.ap()[2 * T : 2 * T + 2].rearrange("g p c s -> p c g s"))

        # ---- projections: qT,kT in [dk(2 heads), pack, l=512]; v natural ----
        qt = qk_p.tile([128, 4, 512], CDT, tag="qt")
        kt = qk_p.tile([128, 4, 512], CDT, tag="kt")
        vE = qk_p.tile([128, 4, HL, 65], CDT, tag="vE")
        for w_sb, dst in ((wq_sb, qt), (wk_sb, kt)):
            for pkk in range(4):
                prj = pp.tile([128, 512], F32, tag="ps")
                for kc in range(8):
                    nc.tensor.matmul(
                        prj[:],
                        w_sb[:, kc, pkk * 128 : (pkk + 1) * 128],
                        xt_sb[:, kc, :, :],
                        start=(kc == 0),
                        stop=(kc == 7),
                    )
                nc.vector.tensor_copy(dst[:, pkk, :], prj[:])
        for c in range(4):
            prj = pp.tile([128, 512], F32, tag="ps")
            for kc in range(8):
                nc.tensor.matmul(
                    prj[:],
                    xt_sb[:, kc, c // 2, (c % 2) * 128 : (c % 2) * 128 + 128],
                    wv_sb[:, kc, :],
                    start=(kc == 0),
                    stop=(kc == 7),
                )
            nc.vector.tensor_copy(vE[:, c, :, 0:64], prj[:].rearrange("p (h j) -> p h j", h=HL))
        nc.vector.memset(vE[:, :, :, 64:65], 1.0)

        # ---- elu(x)+1 = exp(min(x,0)) + max(x,0) ----
        sq = qk_p.tile([128, 4, 512], CDT, tag="sq")
        sk = qk_p.tile([128, 4, 512], F32, tag="sk")
        for src, dst in ((qt, sq), (kt, sk)):
            m0 = qk_p.tile([128, 4, 512], CDT, tag="m0")
            ex = qk_p.tile([128, 4, 512], CDT, tag="ex")
            nc.vector.tensor_scalar_min(m0[:], src[:], 0.0)
            nc.scalar.activation(ex[:], m0[:], AF.Exp)
            nc.vector.scalar_tensor_tensor(dst[:], src[:], 0.0, ex[:], op0=OP.max, op1=OP.add)

        for s in range(2):
            t = 2 * T + s
            so = s * SEG
            # attS: att^T restacked for K=128 output projection.
            # rows [0:64] = att^T[:, l even], rows [64:128] = att^T[:, l odd]
            attS = at_p.tile([128, HL, 128], CDT, tag="attS")

            def stage_a(h):
                hp, pk = h % 2, h // 2
                hs = hp * 64
                qTh = qt[hs : hs + 64, pk, so : so + SEG]
                sqh = sq[hs : hs + 64, pk, so : so + SEG]

                # zBt: z broadcast along free, at this head's partitions
                zBt = sm_p.tile([128, 128], CDT, tag="zBt")
                nc.vector.tensor_scalar_mul(
                    zBt[hs : hs + 64, :], ones128c[hs : hs + 64, :], st32s[pk][hs : hs + 64, 64:65]
                )

                # scoresT[m, l] (2 m-chunks in one bank)
                scT = pa.tile([128, 2, SEG], F32, tag="ps")
                for mc in range(2):
                    nc.tensor.matmul(
                        scT[:, mc, :],
                        kt[hs : hs + 64, pk, so + mc * 128 : so + (mc + 1) * 128],
                        qTh,
                        start=True,
                        stop=True,
                    )
                # P^T = exp(scores/8) (no max subtraction; scores are O(1))
                PT = pt_p.tile([128, 2, SEG], CDT, tag="PT")
                nc.scalar.activation(PT[:], scT[:], AF.Exp, scale=0.125)

                # U: att_memT_raw in [0:64, 256:512]; (dpaT added in stage b)
                UU = pa.tile([128, 512], F32, tag="ps")
                nc.tensor.matmul(UU[0:64, 256:512], stcds[pk][hs : hs + 64, 0:64], sqh, start=True, stop=True)
                # BB: sigma_q @ z broadcast in [0:64, 256:512]; (sum_m P in stage b)
                BB = pa.tile([128, 512], F32, tag="ps")
                nc.tensor.matmul(BB[0:64, 256:512], zBt[hs : hs + 64, 0:64], sqh, start=True, stop=True)

                # sigma_k transposes (PE) + copies to SBUF (ACT)
                trd = pa.tile([128, 256], F32, tag="ps")
                for mc in range(2):
                    nc.tensor.transpose(
                        trd[:, mc * 64 : (mc + 1) * 64],
                        sk[hs : hs + 64, pk, so + mc * 128 : so + (mc + 1) * 128],
                        ident[hs : hs + 64, 0:64],
                    )
                skn = sm_p.tile([128, 2, 64], CDT, tag="skn")
                for mc in range(2):
                    nc.scalar.copy(skn[:, mc, :], trd[:, mc * 64 : (mc + 1) * 64])
                return (h, PT, UU, BB, trd, skn)

            def stage_b(st):
                h, PT, UU, BB, trd, skn = st
                hp, pk = h % 2, h // 2
                hs = hp * 64
                for mc in range(2):
                    nc.tensor.matmul(
                        UU[0:64, 0:256], vE[:, 2 * s + mc, h, 0:64], PT[:, mc, :], start=(mc == 0), stop=(mc == 1)
                    )
                for mc in range(2):
                    nc.tensor.matmul(BB[0:64, 0:256], onesB[:, 0:64], PT[:, mc, :], start=(mc == 0), stop=(mc == 1))
                for mc in range(2):
                    nc.tensor.matmul(
                        trd[hs : hs + 64, 128:193], skn[:, mc, :], vE[:, 2 * s + mc, h, :], start=(mc == 0), stop=(mc == 1)
                    )

                # reciprocal of both denominators: one fast custom-DVE op
                rb2 = sm_p.tile([64, 512], F32, tag="rb2")
                nc.vector.reciprocal_approx_fast(rb2[:], BB[0:64, 0:512])

                # combine: att = bsig * att_mem / zden + (1-bsig) * dpa / sden
                bn = sm_p.tile([64, SEG], F32, tag="bn")
                t2 = sm_p.tile([64, SEG], F32, tag="t2")
                nc.vector.scalar_tensor_tensor(
                    bn[:], UU[0:64, 256:512], bsig_sb[:, h : h + 1], rb2[:, 256:512], op0=OP.mult, op1=OP.mult
                )
                nc.vector.scalar_tensor_tensor(
                    t2[:], UU[0:64, 0:256], bsig1m_sb[:, h : h + 1], rb2[:, 0:256], op0=OP.mult, op1=OP.mult
                )
                bne = bn[:].rearrange("p (a two) -> p a two", two=2)
                t2e = t2[:].rearrange("p (a two) -> p a two", two=2)
                nc.vector.tensor_add(attS[0:64, h, :], bne[:, :, 0], t2e[:, :, 0])
                bo = sm_p.tile([64, 128], CDT, tag="bo")
                nc.gpsimd.tensor_add(bo[:], bne[:, :, 1], t2e[:, :, 1])
                nc.sync.dma_start(attS[64:128, h, :], bo[:])

                # state update: mem += sigma_k^T v ; z += sum_l sigma_k
                nc.vector.tensor_add(st32s[pk][hs : hs + 64, :], st32s[pk][hs : hs + 64, :], trd[hs : hs + 64, 128:193])
                nc.scalar.copy(stcds[pk][hs : hs + 64, :], st32s[pk][hs : hs + 64, :])

            # software-pipelined head loop; order alternates packs so stage_a(h)
            # never reads a state tile stage_b(prev) is writing. The previous
            # segment's output projection is emitted after the first two
            # stage_a's so its PE work lands where the combine tail would
            # otherwise stall the tensor engine.
            horder = [0, 2, 4, 6, 1, 3, 5, 7]
            pending = stage_a(horder[0])
            nxt = stage_a(horder[1])
            if prev_out is not None:
                prev_out()
                prev_out = None
            stage_b(pending)
            pending = nxt
            for h in horder[2:]:
                nxt = stage_a(h)
                stage_b(pending)
                pending = nxt
            stage_b(pending)

            def make_outproj(attS, t):
                def emit():
                    # out_rows = att_view @ Wout (raw-view structure: 8 rank-128 updates)
                    o_sb = ob_p.tile([128, 1024], F32, tag="o")
                    aT = attS[:].rearrange("p h (lh lmo) -> p h lh lmo", lmo=8)
                    for nh in range(2):
                        pot = po.tile([128, 512], F32, tag="ps")
                        for lm0 in range(8):
                            rhs = wout_sb[:, lm0, nh * 512 : (nh + 1) * 512]
                            nc.tensor.matmul(pot[:], aT[:, :, :, lm0], rhs, start=(lm0 == 0), stop=(lm0 == 7))
                        nc.scalar.copy(o_sb[:, nh * 512 : (nh + 1) * 512], pot[:])
                    nc.sync.dma_start(out_d.ap()[t * 128 : (t + 1) * 128, :], o_sb[:])

                return emit

            prev_out = make_outproj(attS, t)
    prev_out()


def build_program(nseg=NSEG):
    nc = bacc.Bacc("TRN2", target_bir_lowering=False, debug=False, num_devices=NCORES)
    xt_d = nc.dram_tensor("xt", [nseg, 128, 8, SEG], CDT, kind="ExternalInput")
    wq_d = nc.dram_tensor("wq", [128, 8, 512], CDT, kind="ExternalInput")
    wk_d = nc.dram_tensor("wk", [128, 8, 512], CDT, kind="ExternalInput")
    wv_d = nc.dram_tensor("wv", [128, 8, 512], CDT, kind="ExternalInput")
    wout_d = nc.dram_tensor("wout", [128, 8, 1024], CDT, kind="ExternalInput")
    bsig_d = nc.dram_tensor("bsig", [64, HL], F32, kind="ExternalInput")
    bsig1m_d = nc.dram_tensor("bsig1m", [64, HL], F32, kind="ExternalInput")
    id64_d = nc.dram_tensor("id64", [128, 64], F32, kind="ExternalInput")
    out_d = nc.dram_tensor("out", [nseg * 128, 1024], F32, kind="ExternalOutput")
    with tile.TileContext(nc) as tc:
        with ExitStack() as ctx:
            _emit(ctx, tc, nseg, xt_d, wq_d, wk_d, wv_d, wout_d, bsig_d, bsig1m_d, id64_d, out_d)
    nc.compile()
    return nc


def shard_inputs(x, Wq, Wk, Wv, Wout, betas, nseg=NSEG):
    x = np.asarray(x, np.float32)
    Wq = np.asarray(Wq, np.float32)
    Wk = np.asarray(Wk, np.float32)
    Wv = np.asarray(Wv, np.float32)
    Wout = np.asarray(Wout, np.float32)
    betas = np.asarray(betas, np.float32)
    sig = 1.0 / (1.0 + np.exp(-betas[0, :, 0, :]))  # [H, dv]

    wout_t = np.ascontiguousarray(Wout.reshape(8, 128, 1024).transpose(1, 0, 2)).astype(NPDT)
    id64 = np.tile(np.eye(64, dtype=np.float32), (2, 1))
    in_maps = []
    for c in range(NCORES):
        b, hg = c // 2, c % 2
        hb = hg * HL
        xt = x[b].T.reshape(8, 128, S // SEG, SEG).transpose(2, 1, 0, 3)[:nseg]
        m = {
            "xt": np.ascontiguousarray(xt).astype(NPDT),
            "wq": np.ascontiguousarray(Wq[:, hb * 64 : (hb + HL) * 64].reshape(8, 128, 512).transpose(1, 0, 2)).astype(NPDT),
            "wk": np.ascontiguousarray(Wk[:, hb * 64 : (hb + HL) * 64].reshape(8, 128, 512).transpose(1, 0, 2)).astype(NPDT),
            "wv": np.ascontiguousarray(Wv[:, hb * 64 : (hb + HL) * 64].reshape(8, 128, 512).transpose(1, 0, 2)).astype(NPDT),
            "wout": wout_t,
            "bsig": np.ascontiguousarray(sig[hb : hb + HL].T),
            "bsig1m": np.ascontiguousarray((1.0 - sig)[hb : hb + HL].T),
            "id64": id64,
        }
        in_maps.append(m)
    return in_maps


def assemble_output(results, nseg=NSEG):
    out = np.empty((B, nseg * SEG, D), np.float32)
    o5 = out.reshape(B, nseg, 2, 128, D)
    for c in range(NCORES):
        b, hg = c // 2, c % 2
        o5[b, :, hg] = results[c]["out"].reshape(nseg, 128, D)
    return out


_COMPILED = {}


def _get_program(nseg=NSEG):
    if nseg not in _COMPILED:
        _COMPILED[nseg] = build_program(nseg)
    return _COMPILED[nseg]


def run(x, Wq, Wk, Wv, Wout, betas, nseg=NSEG, trace=False):
    nc = _get_program(nseg)
    in_maps = shard_inputs(x, Wq, Wk, Wv, Wout, betas, nseg)
    res = run_bass_kernel_spmd(nc, in_maps, list(range(NCORES)), trace=trace)
    return assemble_output(res.results, nseg), res.exec_time_ns


def kernel(x, Wq, Wk, Wv, Wout, betas):
    out, _ = run(x, Wq, Wk, Wv, Wout, betas, nseg=NSEG, trace=False)
    return out


# revision 42
# speedup vs baseline: 2.1302x; 1.0446x over previous
"""Trainium2 Bass kernel for CompressiveMemory (Infini-attention style).

Sharding: 8 cores = 4 batch x 2 head-groups (8 heads each). The reference's
`att.reshape(B, SEG, H*dv)` is a raw view of a (B,H,SEG,dv) tensor, so each
block of 16 output rows depends on exactly one head: head-sharding needs no
cross-core reduction, only row scattering (done on host).

Per-core kernel (all layouts chosen so matmul contractions land on the
partition dim, avoiding transposes except 2 tiny ones per head-segment):
  - host passes x[b] pre-transposed/tiled as xt[t, p, c, s]
  - per segment: project qT,kT (dk-on-partitions) and v (natural), elu+1
  - attention computed transposed: scoresT = kT.T-contracted, exp without
    max-subtraction (scores are O(1): q,k ~ N(0,1), scaled by 1/8), softmax
    denominator via ones-matmul partition reduction
  - memory state [dk, dv+1] per head (fp32 master + compute-dtype shadow)
  - output projection uses the raw-view structure: 16 rank-64 PSUM-accumulated
    matmuls whose lhsT operands are pure AP slices of the transposed att tile
"""

import os
import sys

for _p in ("/opt/trn_rl_repo",):
    if _p not in sys.path and os.path.isdir(_p):
        sys.path.insert(0, _p)

from contextlib import ExitStack

import ml_dtypes
import numpy as np

import concourse.bass as bass
import concourse.tile as tile
from concourse import bacc, mybir
from concourse.bass_utils import run_bass_kernel_spmd

AF = mybir.ActivationFunctionType
OP = mybir.AluOpType
F32 = mybir.dt.float32

B, S, D = 4, 8192, 1024
H, dk, dv, SEG = 16, 64, 64, 256
HL = 8  # heads per core
NCORES = 8

NSEG = int(os.environ.get("BASS_NSEG", S // SEG))
USE_BF16 = os.environ.get("BASS_CDT", "bf16") == "bf16"
CDT = mybir.dt.bfloat16 if USE_BF16 else F32
NPDT = ml_dtypes.bfloat16 if USE_BF16 else np.float32


def _emit(ctx, tc, nseg, xt_d, wq_d, wk_d, wv_d, wout_d, bsig_d, bsig1m_d, id64_d, out_d):
    nc = tc.nc

    consts = ctx.enter_context(tc.tile_pool(name="consts", bufs=1))
    state_p = ctx.enter_context(tc.tile_pool(name="state", bufs=1))
    xt_p = ctx.enter_context(tc.tile_pool(name="xtp", bufs=2))
    qk_p = ctx.enter_context(tc.tile_pool(name="qk", bufs=2))
    pt_p = ctx.enter_context(tc.tile_pool(name="ptp", bufs=4))
    at_p = ctx.enter_context(tc.tile_pool(name="atp", bufs=2))
    sm_p = ctx.enter_context(tc.tile_pool(name="smp", bufs=4))
    ob_p = ctx.enter_context(tc.tile_pool(name="obp", bufs=2))
    ps = ctx.enter_context(tc.tile_pool(name="ps", bufs=8, space="PSUM"))
    pp = pa = po = ps

    wq_sb = consts.tile([128, 8, 512], CDT, tag="wq")
    wk_sb = consts.tile([128, 8, 512], CDT, tag="wk")
    wv_sb = consts.tile([128, 8, 512], CDT, tag="wv")
    wout_sb = consts.tile([128, 8, 1024], CDT, tag="wout")
    nc.sync.dma_start(wq_sb[:], wq_d.ap())
    nc.sync.dma_start(wk_sb[:], wk_d.ap())
    nc.sync.dma_start(wv_sb[:], wv_d.ap())
    nc.sync.dma_start(wout_sb[:], wout_d.ap())
    bsig_sb = consts.tile([64, HL], F32, tag="bsig")
    bsig1m_sb = consts.tile([64, HL], F32, tag="bsig1m")
    nc.sync.dma_start(bsig_sb[:], bsig_d.ap())
    nc.sync.dma_start(bsig1m_sb[:], bsig1m_d.ap())
    ident = consts.tile([128, 64], F32, tag="ident")
    nc.sync.dma_start(ident[:], id64_d.ap())
    onesB = consts.tile([128, 128], CDT, tag="onesB")
    nc.vector.memset(onesB[:], 1.0)
    ones128c = consts.tile([128, 128], CDT, tag="ones128c")
    nc.vector.memset(ones128c[:], 1.0)

    # per-head memory state, 2 heads packed on partitions; one tile per pack
    # (separate tiles so one head-pair's update never serializes another's reads)
    st32s = [state_p.tile([128, 65], F32, tag=f"st32_{i}", name=f"st32_{i}") for i in range(HL // 2)]
    stcds = [state_p.tile([128, 65], CDT, tag=f"stcd_{i}", name=f"stcd_{i}") for i in range(HL // 2)]
    for i in range(HL // 2):
        nc.vector.memset(st32s[i][:], 0.0)
        nc.vector.memset(st32s[i][:, 64:65], 1.0 / dk)
        nc.scalar.copy(stcds[i][:], st32s[i][:])

    assert nseg % 2 == 0
    prev_out = None
    for T in range(nseg // 2):
        # superseg of 2 segments: projections at N=512, halving matmul+ldweights count
        xt_sb = xt_p.tile([128, 8, 2, SEG], CDT, tag="xt")
        nc.sync.dma_start(xt_sb[:], xt_d.ap()[2 * T : 2 * T + 2].rearrange("g p c s -> p c g s"))

        # ---- projections: qT,kT in [dk(2 heads), pack, l=512]; v natural ----
        qt = qk_p.tile([128, 4, 512], CDT, tag="qt")
        kt = qk_p.tile([128, 4, 512], CDT, tag="kt")
        vE = qk_p.tile([128, 4, HL, 65], CDT, tag="vE")
        for w_sb, dst in ((wq_sb, qt), (wk_sb, kt)):
            for pkk in range(4):
                prj = pp.tile([128, 512], F32, tag="ps")
                for kc in range(8):
                    nc.tensor.matmul(
                        prj[:],
                        w_sb[:, kc, pkk * 128 : (pkk + 1) * 128],
                        xt_sb[:, kc, :, :],
                        start=(kc == 0),
                        stop=(kc == 7),
                    )
                (nc.scalar.copy if w_sb is wq_sb else nc.vector.tensor_copy)(dst[:, pkk, :], prj[:])
        for c in range(4):
            prj = pp.tile([128, 512], F32, tag="ps")
            for kc in range(8):
                nc.tensor.matmul(
                    prj[:],
                    xt_sb[:, kc, c // 2, (c % 2) * 128 : (c % 2) * 128 + 128],
                    wv_sb[:, kc, :],
                    start=(kc == 0),
                    stop=(kc == 7),
                )
            nc.vector.tensor_copy(vE[:, c, :, 0:64], prj[:].rearrange("p (h j) -> p h j", h=HL))
        nc.vector.memset(vE[:, :, :, 64:65], 1.0)

        # ---- elu(x)+1 = exp(min(x,0)) + max(x,0) ----
        sq = qk_p.tile([128, 4, 512], CDT, tag="sq")
        sk = qk_p.tile([128, 4, 512], F32, tag="sk")
        for src, dst in ((qt, sq), (kt, sk)):
            m0 = qk_p.tile([128, 4, 512], CDT, tag="m0")
            ex = qk_p.tile([128, 4, 512], CDT, tag="ex")
            nc.vector.tensor_scalar_min(m0[:], src[:], 0.0)
            nc.scalar.activation(ex[:], m0[:], AF.Exp)
            nc.vector.scalar_tensor_tensor(dst[:], src[:], 0.0, ex[:], op0=OP.max, op1=OP.add)

        for s in range(2):
            t = 2 * T + s
            so = s * SEG
            # attS: att^T restacked for K=128 output projection.
            # rows [0:64] = att^T[:, l even], rows [64:128] = att^T[:, l odd]
            attS = at_p.tile([128, HL, 128], CDT, tag="attS")

            def stage_a(h):
                hp, pk = h % 2, h // 2
                hs = hp * 64
                qTh = qt[hs : hs + 64, pk, so : so + SEG]
                sqh = sq[hs : hs + 64, pk, so : so + SEG]

                # zBt: z broadcast along free, at this head's partitions
                zBt = sm_p.tile([128, 128], CDT, tag="zBt")
                nc.scalar.mul(zBt[hs : hs + 64, :], ones128c[hs : hs + 64, :], st32s[pk][hs : hs + 64, 64:65])

                # scoresT[m, l] (2 m-chunks in one bank)
                scT = pa.tile([128, 2, SEG], F32, tag="ps")
                for mc in range(2):
                    nc.tensor.matmul(
                        scT[:, mc, :],
                        kt[hs : hs + 64, pk, so + mc * 128 : so + (mc + 1) * 128],
                        qTh,
                        start=True,
                        stop=True,
                    )
                # P^T = exp(scores/8) (no max subtraction; scores are O(1))
                PT = pt_p.tile([128, 2, SEG], CDT, tag="PT")
                nc.scalar.activation(PT[:], scT[:], AF.Exp, scale=0.125)

                # U: att_memT_raw in [0:64, 256:512]; (dpaT added in stage b)
                UU = pa.tile([128, 512], F32, tag="ps")
                nc.tensor.matmul(UU[0:64, 256:512], stcds[pk][hs : hs + 64, 0:64], sqh, start=True, stop=True)
                # BB: sigma_q @ z broadcast in [0:64, 256:512]; (sum_m P in stage b)
                BB = pa.tile([128, 512], F32, tag="ps")
                nc.tensor.matmul(BB[0:64, 256:512], zBt[hs : hs + 64, 0:64], sqh, start=True, stop=True)

                # sigma_k transposes (PE) + copies to SBUF (ACT)
                trd = pa.tile([128, 256], F32, tag="ps")
                for mc in range(2):
                    nc.tensor.transpose(
                        trd[:, mc * 64 : (mc + 1) * 64],
                        sk[hs : hs + 64, pk, so + mc * 128 : so + (mc + 1) * 128],
                        ident[hs : hs + 64, 0:64],
                    )
                skn = sm_p.tile([128, 2, 64], CDT, tag="skn")
                for mc in range(2):
                    nc.scalar.copy(skn[:, mc, :], trd[:, mc * 64 : (mc + 1) * 64])
                return (h, PT, UU, BB, trd, skn)

            def stage_b(st):
                h, PT, UU, BB, trd, skn = st
                hp, pk = h % 2, h // 2
                hs = hp * 64
                for mc in range(2):
                    nc.tensor.matmul(
                        UU[0:64, 0:256], vE[:, 2 * s + mc, h, 0:64], PT[:, mc, :], start=(mc == 0), stop=(mc == 1)
                    )
                for mc in range(2):
                    nc.tensor.matmul(BB[0:64, 0:256], onesB[:, 0:64], PT[:, mc, :], start=(mc == 0), stop=(mc == 1))
                for mc in range(2):
                    nc.tensor.matmul(
                        trd[hs : hs + 64, 128:193], skn[:, mc, :], vE[:, 2 * s + mc, h, :], start=(mc == 0), stop=(mc == 1)
                    )

                # reciprocal of both denominators: one fast custom-DVE op
                rb2 = sm_p.tile([64, 512], F32, tag="rb2")
                nc.vector.reciprocal_approx_fast(rb2[:], BB[0:64, 0:512])

                # combine: att = bsig * att_mem / zden + (1-bsig) * dpa / sden
                bn = sm_p.tile([64, SEG], F32, tag="bn")
                t2 = sm_p.tile([64, SEG], F32, tag="t2")
                nc.vector.scalar_tensor_tensor(
                    bn[:], UU[0:64, 256:512], bsig_sb[:, h : h + 1], rb2[:, 256:512], op0=OP.mult, op1=OP.mult
                )
                nc.vector.scalar_tensor_tensor(
                    t2[:], UU[0:64, 0:256], bsig1m_sb[:, h : h + 1], rb2[:, 0:256], op0=OP.mult, op1=OP.mult
                )
                bne = bn[:].rearrange("p (a two) -> p a two", two=2)
                t2e = t2[:].rearrange("p (a two) -> p a two", two=2)
                nc.vector.tensor_add(attS[0:64, h, :], bne[:, :, 0], t2e[:, :, 0])
                bo = sm_p.tile([64, 128], CDT, tag="bo")
                nc.gpsimd.tensor_add(bo[:], bne[:, :, 1], t2e[:, :, 1])
                nc.sync.dma_start(attS[64:128, h, :], bo[:])

                # state update: mem += sigma_k^T v ; z += sum_l sigma_k
                nc.vector.tensor_add(st32s[pk][hs : hs + 64, :], st32s[pk][hs : hs + 64, :], trd[hs : hs + 64, 128:193])
                nc.scalar.copy(stcds[pk][hs : hs + 64, :], st32s[pk][hs : hs + 64, :])

            # software-pipelined head loop; order alternates packs so stage_a(h)
            # never reads a state tile stage_b(prev) is writing. The previous
            # segment's output projection is emitted after the first two
            # stage_a's so its PE work lands where the combine tail would
            # otherwise stall the tensor engine.
            horder = [0, 2, 4, 6, 1, 3, 5, 7]
            pending = stage_a(horder[0])
            nxt = stage_a(horder[1])
            if prev_out is not None:
                prev_out()
                prev_out = None
            stage_b(pending)
            pending = nxt
            for h in horder[2:]:
                nxt = stage_a(h)
                stage_b(pending)
                pending = nxt
            stage_b(pending)

            def make_outproj(attS, t):
                def emit():
                    # out_rows = att_view @ Wout (raw-view structure: 8 rank-128 updates)
                    o_sb = ob_p.tile([128, 1024], F32, tag="o")
                    aT = attS[:].rearrange("p h (lh lmo) -> p h lh lmo", lmo=8)
                    for nh in range(2):
                        pot = po.tile([128, 512], F32, tag="ps")
                        for lm0 in range(8):
                            rhs = wout_sb[:, lm0, nh * 512 : (nh + 1) * 512]
                            nc.tensor.matmul(pot[:], aT[:, :, :, lm0], rhs, start=(lm0 == 0), stop=(lm0 == 7))
                        nc.scalar.copy(o_sb[:, nh * 512 : (nh + 1) * 512], pot[:])
                    nc.sync.dma_start(out_d.ap()[t * 128 : (t + 1) * 128, :], o_sb[:])

                return emit

            prev_out = make_outproj(attS, t)
    prev_out()


def build_program(nseg=NSEG):
    nc = bacc.Bacc("TRN2", target_bir_lowering=False, debug=False, num_devices=NCORES)
    xt_d = nc.dram_tensor("xt", [nseg, 128, 8, SEG], CDT, kind="ExternalInput")
    wq_d = nc.dram_tensor("wq", [128, 8, 512], CDT, kind="ExternalInput")
    wk_d = nc.dram_tensor("wk", [128, 8, 512], CDT, kind="ExternalInput")
    wv_d = nc.dram_tensor("wv", [128, 8, 512], CDT, kind="ExternalInput")
    wout_d = nc.dram_tensor("wout", [128, 8, 1024], CDT, kind="ExternalInput")
    bsig_d = nc.dram_tensor("bsig", [64, HL], F32, kind="ExternalInput")
    bsig1m_d = nc.dram_tensor("bsig1m", [64, HL], F32, kind="ExternalInput")
    id64_d = nc.dram_tensor("id64", [128, 64], F32, kind="ExternalInput")
    out_d = nc.dram_tensor("out", [nseg * 128, 1024], F32, kind="ExternalOutput")
    with tile.TileContext(nc) as tc:
        with ExitStack() as ctx:
            _emit(ctx, tc, nseg, xt_d, wq_d, wk_d, wv_d, wout_d, bsig_d, bsig1m_d, id64_d, out_d)
    nc.compile()
    return nc


def shard_inputs(x, Wq, Wk, Wv, Wout, betas, nseg=NSEG):
    x = np.asarray(x, np.float32)
    Wq = np.asarray(Wq, np.float32)
    Wk = np.asarray(Wk, np.float32)
    Wv = np.asarray(Wv, np.float32)
    Wout = np.asarray(Wout, np.float32)
    betas = np.asarray(betas, np.float32)
    sig = 1.0 / (1.0 + np.exp(-betas[0, :, 0, :]))  # [H, dv]

    wout_t = np.ascontiguousarray(Wout.reshape(8, 128, 1024).transpose(1, 0, 2)).astype(NPDT)
    id64 = np.tile(np.eye(64, dtype=np.float32), (2, 1))
    in_maps = []
    for c in range(NCORES):
        b, hg = c // 2, c % 2
        hb = hg * HL
        xt = x[b].T.reshape(8, 128, S // SEG, SEG).transpose(2, 1, 0, 3)[:nseg]
        m = {
            "xt": np.ascontiguousarray(xt).astype(NPDT),
            "wq": np.ascontiguousarray(Wq[:, hb * 64 : (hb + HL) * 64].reshape(8, 128, 512).transpose(1, 0, 2)).astype(NPDT),
            "wk": np.ascontiguousarray(Wk[:, hb * 64 : (hb + HL) * 64].reshape(8, 128, 512).transpose(1, 0, 2)).astype(NPDT),
            "wv": np.ascontiguousarray(Wv[:, hb * 64 : (hb + HL) * 64].reshape(8, 128, 512).transpose(1, 0, 2)).astype(NPDT),
            "wout": wout_t,
            "bsig": np.ascontiguousarray(sig[hb : hb + HL].T),
            "bsig1m": np.ascontiguousarray((1.0 - sig)[hb : hb + HL].T),
            "id64": id64,
        }
        in_maps.append(m)
    return in_maps


def assemble_output(results, nseg=NSEG):
    out = np.empty((B, nseg * SEG, D), np.float32)
    o5 = out.reshape(B, nseg, 2, 128, D)
    for c in range(NCORES):
        b, hg = c // 2, c % 2
        o5[b, :, hg] = results[c]["out"].reshape(nseg, 128, D)
    return out


_COMPILED = {}


def _get_program(nseg=NSEG):
    if nseg not in _COMPILED:
        _COMPILED[nseg] = build_program(nseg)
    return _COMPILED[nseg]


def run(x, Wq, Wk, Wv, Wout, betas, nseg=NSEG, trace=False):
    nc = _get_program(nseg)
    in_maps = shard_inputs(x, Wq, Wk, Wv, Wout, betas, nseg)
    res = run_bass_kernel_spmd(nc, in_maps, list(range(NCORES)), trace=trace)
    return assemble_output(res.results, nseg), res.exec_time_ns


def kernel(x, Wq, Wk, Wv, Wout, betas):
    out, _ = run(x, Wq, Wk, Wv, Wout, betas, nseg=NSEG, trace=False)
    return out


# revision 43
# speedup vs baseline: 2.5570x; 1.2003x over previous
"""Trainium2 Bass kernel for CompressiveMemory (Infini-attention style).

Sharding: 8 cores = 4 batch x 2 head-groups (8 heads each). The reference's
`att.reshape(B, SEG, H*dv)` is a raw view of a (B,H,SEG,dv) tensor, so each
block of 16 output rows depends on exactly one head: head-sharding needs no
cross-core reduction, only row scattering (done on host).

Per-core kernel (all layouts chosen so matmul contractions land on the
partition dim, avoiding transposes except 2 tiny ones per head-segment):
  - host passes x[b] pre-transposed/tiled as xt[t, p, c, s]
  - per segment: project qT,kT (dk-on-partitions) and v (natural), elu+1
  - attention computed transposed: scoresT = kT.T-contracted, exp without
    max-subtraction (scores are O(1): q,k ~ N(0,1), scaled by 1/8), softmax
    denominator via ones-matmul partition reduction
  - memory state [dk, dv+1] per head (fp32 master + compute-dtype shadow)
  - output projection uses the raw-view structure: 16 rank-64 PSUM-accumulated
    matmuls whose lhsT operands are pure AP slices of the transposed att tile
"""

import os
import sys

for _p in ("/opt/trn_rl_repo",):
    if _p not in sys.path and os.path.isdir(_p):
        sys.path.insert(0, _p)

from contextlib import ExitStack

import ml_dtypes
import numpy as np

import concourse.bass as bass
import concourse.tile as tile
from concourse import bacc, mybir
from concourse.bass_utils import run_bass_kernel_spmd

AF = mybir.ActivationFunctionType
OP = mybir.AluOpType
F32 = mybir.dt.float32

B, S, D = 4, 8192, 1024
H, dk, dv, SEG = 16, 64, 64, 256
HL = 8  # heads per core
NCORES = 8

NSEG = int(os.environ.get("BASS_NSEG", S // SEG))
USE_BF16 = os.environ.get("BASS_CDT", "bf16") == "bf16"
CDT = mybir.dt.bfloat16 if USE_BF16 else F32
NPDT = ml_dtypes.bfloat16 if USE_BF16 else np.float32


def _emit(ctx, tc, nseg, xt_d, wq_d, wk_d, wv_d, wout_d, bsig_d, bsig1m_d, id64_d, out_d):
    nc = tc.nc

    consts = ctx.enter_context(tc.tile_pool(name="consts", bufs=1))
    state_p = ctx.enter_context(tc.tile_pool(name="state", bufs=1))
    xt_p = ctx.enter_context(tc.tile_pool(name="xtp", bufs=2))
    qk_p = ctx.enter_context(tc.tile_pool(name="qk", bufs=2))
    pt_p = ctx.enter_context(tc.tile_pool(name="ptp", bufs=4))
    at_p = ctx.enter_context(tc.tile_pool(name="atp", bufs=2))
    sm_p = ctx.enter_context(tc.tile_pool(name="smp", bufs=4))
    ob_p = ctx.enter_context(tc.tile_pool(name="obp", bufs=2))
    ps = ctx.enter_context(tc.tile_pool(name="ps", bufs=8, space="PSUM"))
    pp = pa = po = ps

    wq_sb = consts.tile([128, 8, 512], CDT, tag="wq")
    wk_sb = consts.tile([128, 8, 512], CDT, tag="wk")
    wv_sb = consts.tile([128, 8, 512], CDT, tag="wv")
    wout_sb = consts.tile([128, 8, 1024], CDT, tag="wout")
    nc.sync.dma_start(wq_sb[:], wq_d.ap())
    nc.sync.dma_start(wk_sb[:], wk_d.ap())
    nc.sync.dma_start(wv_sb[:], wv_d.ap())
    nc.sync.dma_start(wout_sb[:], wout_d.ap())
    bsig_sb = consts.tile([64, HL], F32, tag="bsig")
    bsig1m_sb = consts.tile([64, HL], F32, tag="bsig1m")
    nc.sync.dma_start(bsig_sb[:], bsig_d.ap())
    nc.sync.dma_start(bsig1m_sb[:], bsig1m_d.ap())
    ident = consts.tile([128, 64], CDT, tag="ident")
    nc.sync.dma_start(ident[:], id64_d.ap())
    onesB = consts.tile([128, 128], CDT, tag="onesB")
    nc.vector.memset(onesB[:], 1.0)
    ones128c = consts.tile([128, 128], CDT, tag="ones128c")
    nc.vector.memset(ones128c[:], 1.0)

    # per-head memory state, 2 heads packed on partitions; one tile per pack
    # (separate tiles so one head-pair's update never serializes another's reads)
    st32s = [state_p.tile([128, 65], F32, tag=f"st32_{i}", name=f"st32_{i}") for i in range(HL // 2)]
    stcds = [state_p.tile([128, 65], CDT, tag=f"stcd_{i}", name=f"stcd_{i}") for i in range(HL // 2)]
    for i in range(HL // 2):
        nc.vector.memset(st32s[i][:], 0.0)
        nc.vector.memset(st32s[i][:, 64:65], 1.0 / dk)
        nc.scalar.copy(stcds[i][:], st32s[i][:])

    assert nseg % 2 == 0
    prev_out = None
    for T in range(nseg // 2):
        # superseg of 2 segments: projections at N=512, halving matmul+ldweights count
        xt_sb = xt_p.tile([128, 8, 2, SEG], CDT, tag="xt")
        nc.sync.dma_start(xt_sb[:], xt_d.ap()[2 * T : 2 * T + 2].rearrange("g p c s -> p c g s"))

        # ---- projections: qT,kT in [dk(2 heads), pack, l=512]; v natural ----
        qt = qk_p.tile([128, 4, 512], CDT, tag="qt")
        kt = qk_p.tile([128, 4, 512], CDT, tag="kt")
        vE = qk_p.tile([128, 4, HL, 65], CDT, tag="vE")
        for w_sb, dst in ((wq_sb, qt), (wk_sb, kt)):
            for pkk in range(4):
                prj = pp.tile([128, 512], F32, tag="ps")
                for kc in range(8):
                    nc.tensor.matmul(
                        prj[:],
                        w_sb[:, kc, pkk * 128 : (pkk + 1) * 128],
                        xt_sb[:, kc, :, :],
                        start=(kc == 0),
                        stop=(kc == 7),
                    )
                (nc.scalar.copy if w_sb is wq_sb else nc.vector.tensor_copy)(dst[:, pkk, :], prj[:])
        for c in range(4):
            prj = pp.tile([128, 512], F32, tag="ps")
            for kc in range(8):
                nc.tensor.matmul(
                    prj[:],
                    xt_sb[:, kc, c // 2, (c % 2) * 128 : (c % 2) * 128 + 128],
                    wv_sb[:, kc, :],
                    start=(kc == 0),
                    stop=(kc == 7),
                )
            nc.vector.tensor_copy(vE[:, c, :, 0:64], prj[:].rearrange("p (h j) -> p h j", h=HL))
        nc.vector.memset(vE[:, :, :, 64:65], 1.0)

        # ---- elu(x)+1 = exp(min(x,0)) + max(x,0) ----
        sq = qk_p.tile([128, 4, 512], CDT, tag="sq")
        sk = qk_p.tile([128, 4, 512], CDT, tag="sk")
        for src, dst in ((qt, sq), (kt, sk)):
            m0 = qk_p.tile([128, 4, 512], CDT, tag="m0")
            ex = qk_p.tile([128, 4, 512], CDT, tag="ex")
            nc.vector.tensor_scalar_min(m0[:], src[:], 0.0)
            nc.scalar.activation(ex[:], m0[:], AF.Exp)
            nc.vector.scalar_tensor_tensor(dst[:], src[:], 0.0, ex[:], op0=OP.max, op1=OP.add)

        for s in range(2):
            t = 2 * T + s
            so = s * SEG
            # attS: att^T restacked for K=128 output projection.
            # rows [0:64] = att^T[:, l even], rows [64:128] = att^T[:, l odd]
            attS = at_p.tile([128, HL, 128], CDT, tag="attS")

            def stage_a(h):
                hp, pk = h % 2, h // 2
                hs = hp * 64
                qTh = qt[hs : hs + 64, pk, so : so + SEG]
                sqh = sq[hs : hs + 64, pk, so : so + SEG]

                # zBt: z broadcast along free, at this head's partitions
                zBt = sm_p.tile([128, 128], CDT, tag="zBt")
                nc.scalar.mul(zBt[hs : hs + 64, :], ones128c[hs : hs + 64, :], st32s[pk][hs : hs + 64, 64:65])

                # scoresT[m, l] (2 m-chunks in one bank)
                scT = pa.tile([128, 2, SEG], F32, tag="ps")
                for mc in range(2):
                    nc.tensor.matmul(
                        scT[:, mc, :],
                        kt[hs : hs + 64, pk, so + mc * 128 : so + (mc + 1) * 128],
                        qTh,
                        start=True,
                        stop=True,
                    )
                # P^T = exp(scores/8) (no max subtraction; scores are O(1))
                PT = pt_p.tile([128, 2, SEG], CDT, tag="PT")
                nc.scalar.activation(PT[:], scT[:], AF.Exp, scale=0.125)

                # U: att_memT_raw in [0:64, 256:512]; (dpaT added in stage b)
                UU = pa.tile([128, 512], F32, tag="ps")
                nc.tensor.matmul(UU[0:64, 256:512], stcds[pk][hs : hs + 64, 0:64], sqh, start=True, stop=True)
                # BB: sigma_q @ z broadcast in [0:64, 256:512]; (sum_m P in stage b)
                BB = pa.tile([128, 512], F32, tag="ps")
                nc.tensor.matmul(BB[0:64, 256:512], zBt[hs : hs + 64, 0:64], sqh, start=True, stop=True)

                # sigma_k transposes (PE) + copies to SBUF (ACT)
                trd = pa.tile([128, 128], CDT, tag="ps")
                for mc in range(2):
                    nc.tensor.transpose(
                        trd[:, mc * 64 : (mc + 1) * 64],
                        sk[hs : hs + 64, pk, so + mc * 128 : so + (mc + 1) * 128],
                        ident[hs : hs + 64, 0:64],
                    )
                skn = sm_p.tile([128, 2, 64], CDT, tag="skn")
                for mc in range(2):
                    nc.scalar.copy(skn[:, mc, :], trd[:, mc * 64 : (mc + 1) * 64])
                return (h, PT, UU, BB, trd, skn)

            def stage_b(st):
                h, PT, UU, BB, trd, skn = st
                hp, pk = h % 2, h // 2
                hs = hp * 64
                for mc in range(2):
                    nc.tensor.matmul(
                        UU[0:64, 0:256], vE[:, 2 * s + mc, h, 0:64], PT[:, mc, :], start=(mc == 0), stop=(mc == 1)
                    )
                for mc in range(2):
                    nc.tensor.matmul(BB[0:64, 0:256], onesB[:, 0:64], PT[:, mc, :], start=(mc == 0), stop=(mc == 1))
                dm = pa.tile([128, 65], F32, tag="ps", name="dm")
                for mc in range(2):
                    nc.tensor.matmul(
                        dm[hs : hs + 64, :], skn[:, mc, :], vE[:, 2 * s + mc, h, :], start=(mc == 0), stop=(mc == 1)
                    )

                # reciprocal of both denominators: one fast custom-DVE op
                rb2 = sm_p.tile([64, 512], F32, tag="rb2")
                nc.vector.reciprocal_approx_fast(rb2[:], BB[0:64, 0:512])

                # combine: att = bsig * att_mem / zden + (1-bsig) * dpa / sden
                bn = sm_p.tile([64, SEG], F32, tag="bn")
                t2 = sm_p.tile([64, SEG], F32, tag="t2")
                nc.vector.scalar_tensor_tensor(
                    bn[:], UU[0:64, 256:512], bsig_sb[:, h : h + 1], rb2[:, 256:512], op0=OP.mult, op1=OP.mult
                )
                nc.vector.scalar_tensor_tensor(
                    t2[:], UU[0:64, 0:256], bsig1m_sb[:, h : h + 1], rb2[:, 0:256], op0=OP.mult, op1=OP.mult
                )
                bne = bn[:].rearrange("p (a two) -> p a two", two=2)
                t2e = t2[:].rearrange("p (a two) -> p a two", two=2)
                nc.vector.tensor_add(attS[0:64, h, :], bne[:, :, 0], t2e[:, :, 0])
                bo = sm_p.tile([64, 128], CDT, tag="bo")
                nc.gpsimd.tensor_add(bo[:], bne[:, :, 1], t2e[:, :, 1])
                nc.sync.dma_start(attS[64:128, h, :], bo[:])

                # state update: mem += sigma_k^T v ; z += sum_l sigma_k
                nc.vector.tensor_add(st32s[pk][hs : hs + 64, :], st32s[pk][hs : hs + 64, :], dm[hs : hs + 64, :])
                nc.scalar.copy(stcds[pk][hs : hs + 64, :], st32s[pk][hs : hs + 64, :])

            # software-pipelined head loop; order alternates packs so stage_a(h)
            # never reads a state tile stage_b(prev) is writing. The previous
            # segment's output projection is emitted after the first two
            # stage_a's so its PE work lands where the combine tail would
            # otherwise stall the tensor engine.
            horder = [0, 2, 4, 6, 1, 3, 5, 7]
            pending = stage_a(horder[0])
            nxt = stage_a(horder[1])
            if prev_out is not None:
                prev_out()
                prev_out = None
            stage_b(pending)
            pending = nxt
            for h in horder[2:]:
                nxt = stage_a(h)
                stage_b(pending)
                pending = nxt
            stage_b(pending)

            def make_outproj(attS, t):
                def emit():
                    # out_rows = att_view @ Wout (raw-view structure: 8 rank-128 updates)
                    o_sb = ob_p.tile([128, 1024], F32, tag="o")
                    aT = attS[:].rearrange("p h (lh lmo) -> p h lh lmo", lmo=8)
                    for nh in range(2):
                        pot = po.tile([128, 512], F32, tag="ps")
                        for lm0 in range(8):
                            rhs = wout_sb[:, lm0, nh * 512 : (nh + 1) * 512]
                            nc.tensor.matmul(pot[:], aT[:, :, :, lm0], rhs, start=(lm0 == 0), stop=(lm0 == 7))
                        nc.scalar.copy(o_sb[:, nh * 512 : (nh + 1) * 512], pot[:])
                    nc.sync.dma_start(out_d.ap()[t * 128 : (t + 1) * 128, :], o_sb[:])

                return emit

            prev_out = make_outproj(attS, t)
    prev_out()


def build_program(nseg=NSEG):
    nc = bacc.Bacc("TRN2", target_bir_lowering=False, debug=False, num_devices=NCORES)
    xt_d = nc.dram_tensor("xt", [nseg, 128, 8, SEG], CDT, kind="ExternalInput")
    wq_d = nc.dram_tensor("wq", [128, 8, 512], CDT, kind="ExternalInput")
    wk_d = nc.dram_tensor("wk", [128, 8, 512], CDT, kind="ExternalInput")
    wv_d = nc.dram_tensor("wv", [128, 8, 512], CDT, kind="ExternalInput")
    wout_d = nc.dram_tensor("wout", [128, 8, 1024], CDT, kind="ExternalInput")
    bsig_d = nc.dram_tensor("bsig", [64, HL], F32, kind="ExternalInput")
    bsig1m_d = nc.dram_tensor("bsig1m", [64, HL], F32, kind="ExternalInput")
    id64_d = nc.dram_tensor("id64", [128, 64], CDT, kind="ExternalInput")
    out_d = nc.dram_tensor("out", [nseg * 128, 1024], F32, kind="ExternalOutput")
    with tile.TileContext(nc) as tc:
        with ExitStack() as ctx:
            _emit(ctx, tc, nseg, xt_d, wq_d, wk_d, wv_d, wout_d, bsig_d, bsig1m_d, id64_d, out_d)
    nc.compile()
    return nc


def shard_inputs(x, Wq, Wk, Wv, Wout, betas, nseg=NSEG):
    x = np.asarray(x, np.float32)
    Wq = np.asarray(Wq, np.float32)
    Wk = np.asarray(Wk, np.float32)
    Wv = np.asarray(Wv, np.float32)
    Wout = np.asarray(Wout, np.float32)
    betas = np.asarray(betas, np.float32)
    sig = 1.0 / (1.0 + np.exp(-betas[0, :, 0, :]))  # [H, dv]

    wout_t = np.ascontiguousarray(Wout.reshape(8, 128, 1024).transpose(1, 0, 2)).astype(NPDT)
    id64 = np.ascontiguousarray(np.tile(np.eye(64, dtype=np.float32), (2, 1))).astype(NPDT)
    in_maps = []
    for c in range(NCORES):
        b, hg = c // 2, c % 2
        hb = hg * HL
        xt = x[b].T.reshape(8, 128, S // SEG, SEG).transpose(2, 1, 0, 3)[:nseg]
        m = {
            "xt": np.ascontiguousarray(xt).astype(NPDT),
            "wq": np.ascontiguousarray(Wq[:, hb * 64 : (hb + HL) * 64].reshape(8, 128, 512).transpose(1, 0, 2)).astype(NPDT),
            "wk": np.ascontiguousarray(Wk[:, hb * 64 : (hb + HL) * 64].reshape(8, 128, 512).transpose(1, 0, 2)).astype(NPDT),
            "wv": np.ascontiguousarray(Wv[:, hb * 64 : (hb + HL) * 64].reshape(8, 128, 512).transpose(1, 0, 2)).astype(NPDT),
            "wout": wout_t,
            "bsig": np.ascontiguousarray(sig[hb : hb + HL].T),
            "bsig1m": np.ascontiguousarray((1.0 - sig)[hb : hb + HL].T),
            "id64": id64,
        }
        in_maps.append(m)
    return in_maps


def assemble_output(results, nseg=NSEG):
    out = np.empty((B, nseg * SEG, D), np.float32)
    o5 = out.reshape(B, nseg, 2, 128, D)
    for c in range(NCORES):
        b, hg = c // 2, c % 2
        o5[b, :, hg] = results[c]["out"].reshape(nseg, 128, D)
    return out


_COMPILED = {}


def _get_program(nseg=NSEG):
    if nseg not in _COMPILED:
        _COMPILED[nseg] = build_program(nseg)
    return _COMPILED[nseg]


def run(x, Wq, Wk, Wv, Wout, betas, nseg=NSEG, trace=False):
    nc = _get_program(nseg)
    in_maps = shard_inputs(x, Wq, Wk, Wv, Wout, betas, nseg)
    res = run_bass_kernel_spmd(nc, in_maps, list(range(NCORES)), trace=trace)
    return assemble_output(res.results, nseg), res.exec_time_ns


def kernel(x, Wq, Wk, Wv, Wout, betas):
    out, _ = run(x, Wq, Wk, Wv, Wout, betas, nseg=NSEG, trace=False)
    return out
